# revision 16
# baseline (speedup 1.0000x reference)
"""Trainium2 Bass kernel for Restormer-style transposed (channel) attention, v2.

Per-core (1 of 8 batch elements), built around the TimelineSim cost model
(matmul cost = output free-size; fp8e4m3 DoubleRow = 0.5 cyc/col):

  q/k path (errors wash out through the softmax normalization):
    z_qk = Wqk8 @ x8            fp8 DoubleRow, 192-contraction in 1 instr
    dwconv 3x3                  5 DR diag tap-pair matmuls per block (2.5 cyc/px)
    -> bf16 qb -> xbar DMA transpose -> [px, slot] qki tiles
    gram G += qki^T qki         bf16, compact-col strided rhs
  v path (kept accurate):
    z_v = Wv @ x16              bf16
    dwconv = DR(fp8(z_v)) + DR(fp8(z_v - fp8(z_v)))   exact to ~0.2%
    vout fp16 resident in SBUF
  tail: norms from gram diag, softmax per head, M^T = A_bd^T Wproj^T,
        out = M @ vout streamed to HBM.

Slot layout (32-aligned, 4 blocks of 128):
  block b: [q_{2b} 0:24 | pad | k_{2b} 32:56 | pad | q_{2b+1} 64:88 | pad |
            k_{2b+1} 96:120 | pad]
"""
import numpy as np

NUM_HEADS = 8
C = 192
H = W = 128
HW = H * W
CD = 24
NCORES = 8
SLAB = 16
NSLABS = H // SLAB
EPS = 1e-12
PW = W + 4
IMG0 = 2
NR = SLAB + 2          # z8 slab rows incl halo

# tap order chosen so DR pairs have EVEN offset deltas (hw requirement):
# pairs: ((-1,-1),(-1,1)) ((0,-1),(0,1)) ((1,-1),(1,1)) ((-1,0),(0,0)) ((1,0),zero)
TAPS = [(-1, -1), (-1, 1), (0, -1), (0, 1), (1, -1), (1, 1), (-1, 0), (0, 0), (1, 0)]

# per-block slot groups: (slot_base, qkv_ch_base)
def _slot_groups(b):
    return [(0, 48 * b), (32, 192 + 48 * b), (64, 48 * b + 24), (96, 192 + 48 * b + 24)]


_CACHE = {}


def _stripes(ncols):
    """split ncols into row-aligned stripes of >=256 (multiples of 128)."""
    out = []
    rem = ncols
    while rem > 0:
        t = min(512, rem)
        if rem - t == 128:
            t = 384
        out.append(t)
        rem -= t
    return out


def _build():
    import concourse.bass as bass
    import concourse.mybir as mybir
    import concourse.tile as tile
    from concourse import bacc
    from contextlib import ExitStack
    import bass_rust

    dt = mybir.dt
    A = mybir.AluOpType
    AF = mybir.ActivationFunctionType
    AX = mybir.AxisListType
    DR = mybir.MatmulPerfMode.DoubleRow
    f32, bf16, f16, f8, f32r = dt.float32, dt.bfloat16, dt.float16, dt.float8e4, dt.float32r

    def ap_dims(ap, dims, extra_offset=0):
        c = ap.copy()
        c.ap = bass_rust.VecI64Pair(dims)
        c.offset = ap.offset + extra_offset
        return c

    nc = bacc.Bacc("TRN2", num_devices=NCORES)

    x8d = nc.dram_tensor("x8", [C, HW], f8, kind="ExternalInput").ap()
    xr8d = nc.dram_tensor("xr8", [C, HW], f8, kind="ExternalInput").ap()
    wq8d = nc.dram_tensor("wq8", [128, 4 * 2 * 128], f8, kind="ExternalInput").ap()
    dwq8d = nc.dram_tensor("dwq8", [128, 4 * 5 * 2 * 128], f8, kind="ExternalInput").ap()
    # v 1x1 weights, out-blocks [128]+[64]: [.., 0:256]=lo j2 m128, [.., 256:384]=hi j2 m64
    wv8d = nc.dram_tensor("wv8", [128, 2 * 128 + 2 * 64], f8, kind="ExternalInput").ap()
    wvr8d = nc.dram_tensor("wvr8", [128, 2 * 128 + 2 * 64], f8, kind="ExternalInput").ap()
    # v dw: A = dual-weight (w8,wr) per tap for ch0-127; rA = tap-paired w8 for r ch0-127;
    # B = mixed block (rows 0-63: z8v ch128-191 dual; rows 64-127: r ch128-191 single w8)
    dwvA8d = nc.dram_tensor("dwvA8", [128, 9 * 2 * 128], f8, kind="ExternalInput").ap()
    dwvrA8d = nc.dram_tensor("dwvrA8", [128, 5 * 2 * 128], f8, kind="ExternalInput").ap()
    dwvB8d = nc.dram_tensor("dwvB8", [128, 9 * 2 * 64], f8, kind="ExternalInput").ap()
    wpTd = nc.dram_tensor("wpT", [C, C], f32, kind="ExternalInput").ap()
    mskd = nc.dram_tensor("gmask", [128, 4 * 96], f32, kind="ExternalInput").ap()
    tmpd = nc.dram_tensor("tmap", [128, 4], f32, kind="ExternalInput").ap()
    eyed = nc.dram_tensor("eye", [128, 24], f32, kind="ExternalInput").ap()
    outd = nc.dram_tensor("out", [C, HW], f16, kind="ExternalOutput").ap()

    with tile.TileContext(nc) as tc:
      with ExitStack() as _es:
        cpool = _es.enter_context(tc.tile_pool(name="const", bufs=1))
        xpool = _es.enter_context(tc.tile_pool(name="xin", bufs=2))
        zpool = _es.enter_context(tc.tile_pool(name="zst", bufs=2))
        qpool = _es.enter_context(tc.tile_pool(name="qbt", bufs=2))
        kpool = _es.enter_context(tc.tile_pool(name="qki", bufs=2))
        vpool = _es.enter_context(tc.tile_pool(name="vout", bufs=1))
        mpool = _es.enter_context(tc.tile_pool(name="sm", bufs=2))
        apool = _es.enter_context(tc.tile_pool(name="abd", bufs=1))
        opool = _es.enter_context(tc.tile_pool(name="outs", bufs=3))
        psP = _es.enter_context(tc.tile_pool(name="psP", bufs=7, space="PSUM"))
        psG = _es.enter_context(tc.tile_pool(name="psG", bufs=1, space="PSUM"))

        def pstile():
            return psP.tile([128, 512], f32, tag="ps", name="ps")

        # ---------- constants (ACT dma queue; x streams on sync) ----------
        wq8 = cpool.tile([128, 4 * 2 * 128], f8, tag="wq8")
        nc.scalar.dma_start(wq8[:, :], wq8d[:, :])
        wv8 = cpool.tile([128, 2 * 128 + 2 * 64], f8, tag="wv8")
        nc.scalar.dma_start(wv8[:, :], wv8d[:, :])
        wvr8 = cpool.tile([128, 2 * 128 + 2 * 64], f8, tag="wvr8")
        nc.scalar.dma_start(wvr8[:, :], wvr8d[:, :])
        dwq8 = cpool.tile([128, 4 * 5 * 2 * 128], f8, tag="dwq8")
        nc.scalar.dma_start(dwq8[:, :], dwq8d[:, :])
        dwvA8 = cpool.tile([128, 9 * 2 * 128], f8, tag="dwvA8")
        nc.scalar.dma_start(dwvA8[:, :], dwvA8d[:, :])
        dwvrA8 = cpool.tile([128, 5 * 2 * 128], f8, tag="dwvrA8")
        nc.scalar.dma_start(dwvrA8[:, :], dwvrA8d[:, :])
        dwvB8 = cpool.tile([128, 9 * 2 * 64], f8, tag="dwvB8")
        nc.scalar.dma_start(dwvB8[:, :], dwvB8d[:, :])
        msk = cpool.tile([128, 4 * 96], f32, tag="msk")
        nc.scalar.dma_start(msk[:, :], mskd[:, :])
        tmap = cpool.tile([128, 4], f32, tag="tmap")
        nc.scalar.dma_start(tmap[:, :], tmpd[:, :])
        eye = cpool.tile([128, 24], f32, tag="eye")
        nc.scalar.dma_start(eye[:, :], eyed[:, :])
        wp0 = cpool.tile([96, C], f32, tag="wp0")
        nc.scalar.dma_start(wp0[:, :], wpTd[0:96, :])
        wp1 = cpool.tile([96, C], f32, tag="wp1")
        nc.scalar.dma_start(wp1[:, :], wpTd[96:192, :])

        wq8v = wq8[:, :].rearrange("p (b j s) -> p b j s", b=4, j=2)
        dwq8v = dwq8[:, :].rearrange("p (b k j s) -> p b k j s", b=4, k=5, j=2)
        wv8lo = wv8[:, 0:256].rearrange("p (j s) -> p j s", j=2)
        wv8hi = wv8[:, 256:384].rearrange("p (j s) -> p j s", j=2)
        wvr8lo = wvr8[:, 0:256].rearrange("p (j s) -> p j s", j=2)
        wvr8hi = wvr8[:, 256:384].rearrange("p (j s) -> p j s", j=2)
        dwvA8v = dwvA8[:, :].rearrange("p (t j s) -> p t j s", t=9, j=2)
        dwvrA8v = dwvrA8[:, :].rearrange("p (t j s) -> p t j s", t=5, j=2)
        dwvB8v = dwvB8[:, :].rearrange("p (t j s) -> p t j s", t=9, j=2)

        # vout in fp8 main+residual; contraction j-split 128+64 for M@v DR
        # (j1 rows 64-127 are zero pad, memset once on Pool)
        vout8 = vpool.tile([128, 2 * HW], f8, tag="vout8", name="vout8")
        voutr8 = vpool.tile([128, 2 * HW], f8, tag="voutr8", name="voutr8")
        nc.gpsimd.memset(vout8[64:128, HW:2 * HW], 0.0)
        nc.gpsimd.memset(voutr8[64:128, HW:2 * HW], 0.0)
        gram = psG.tile([128, 4 * 96], f32, tag="g")

        # evac engine round-robin (ACT / DVE alternating)
        _ev = [0]
        def cpy(dst, src):
            _ev[0] += 1
            if _ev[0] % 2 == 0:
                nc.scalar.copy(dst, src)
            else:
                nc.vector.tensor_copy(dst, src)

        # slab state carried across pipeline iterations
        z8s, zv8s, qbs, qkis, xts = {}, {}, {}, {}, {}

        def _slabmeta(s):
            r0 = SLAB * s
            lo, hi = max(0, r0 - 1), min(H - 1, r0 + SLAB)
            nrows = hi - lo + 1
            return r0, lo, nrows, lo - (r0 - 1)

        def emit_loads(s):
            r0, lo, nrows, slot0 = _slabmeta(s)
            ncols = nrows * W
            col0 = lo * W
            xs8 = xpool.tile([128, 2 * ncols], f8, tag="xs8")
            nc.sync.dma_start(xs8[:, 0:ncols], x8d[0:128, col0:col0 + ncols])
            # k-tile 1 holds x channels 64..191 (rows 0-63 have zero weights)
            nc.sync.dma_start(xs8[:, ncols:2 * ncols],
                              x8d[64:192, col0:col0 + ncols])
            xsr8 = xpool.tile([128, 2 * ncols], f8, tag="xsr8")
            nc.sync.dma_start(xsr8[:, 0:ncols], xr8d[0:128, col0:col0 + ncols])
            nc.sync.dma_start(xsr8[:, ncols:2 * ncols],
                              xr8d[64:192, col0:col0 + ncols])
            xts[s] = (xs8, xsr8, ncols, slot0)

        def emit_qkv_chunk(s, c):
            r0, lo, nrows, slot0 = _slabmeta(s)
            ncols = nrows * W
            if c == 0:
                z8 = [zpool.tile([128, NR * PW], f8, tag=f"z8_{b}", name=f"z8_{b}")
                      for b in range(4)]
                # zA: z8v ch0-127; rA: r ch0-127; B: rows 0-63 z8v ch128-191,
                # rows 64-127 r ch128-191
                zv8 = [zpool.tile([128, NR * PW], f8, tag=f"zv8_{v}", name=f"zv8_{v}")
                       for v in range(3)]
                if s < 2:
                    for t in z8 + zv8:
                        tv = t[:, :].rearrange("p (r w) -> p r w", w=PW)
                        nc.gpsimd.memset(tv[:, :, 0:IMG0], 0.0)
                        nc.gpsimd.memset(tv[:, :, IMG0 + W:PW], 0.0)
                if s == 0:
                    for t in z8 + zv8:
                        nc.gpsimd.memset(t[:, 0:PW], 0.0)
                if s == NSLABS - 1:
                    for t in z8 + zv8:
                        nc.gpsimd.memset(t[:, (NR - 1) * PW:NR * PW], 0.0)
                z8s[s], zv8s[s] = z8, zv8
            xs8, xsr8, ncols, slot0 = xts[s]
            z8, zv8 = z8s[s], zv8s[s]
            zA, rA, zB = zv8
            xs8p = xs8[:, :].ap[0][0]
            xsr8p = xsr8[:, :].ap[0][0]
            strs = _stripes(ncols)
            todo = [c] if c < 3 else [3] + list(range(4, len(strs)))
            for si in todo:
                tw = strs[si]
                t0 = sum(strs[:si])
                row0 = t0 // W
                nr = tw // W
                for b in range(4):
                  with nc.named_scope(f"qkvqk{s}"):
                    ps = pstile()
                    rhs = ap_dims(xs8[:, :], [[xs8p, 128], [ncols, 2], [1, tw]],
                                  extra_offset=t0)
                    nc.tensor.matmul(ps[0:128, 0:tw], wq8v[:, b, :, :], rhs,
                                     start=True, stop=True, perf_mode=DR)
                    zview = z8[b][:, :].rearrange("p (r w) -> p r w", w=PW)
                    cpy(zview[:, slot0 + row0: slot0 + row0 + nr, IMG0:IMG0 + W],
                        ps[0:128, 0:tw].rearrange("p (r w) -> p r w", w=W))
                with nc.named_scope(f"qkvv{s}"):
                    rhs8 = ap_dims(xs8[:, :], [[xs8p, 128], [ncols, 2], [1, tw]],
                                   extra_offset=t0)
                    rhsr = ap_dims(xsr8[:, :], [[xsr8p, 128], [ncols, 2], [1, tw]],
                                   extra_offset=t0)
                    psl = pstile()
                    nc.tensor.matmul(psl[:, 0:tw], wv8lo, rhs8,
                                     start=True, stop=False, perf_mode=DR)
                    nc.tensor.matmul(psl[:, 0:tw], wv8lo, rhsr,
                                     start=False, stop=False, perf_mode=DR)
                    nc.tensor.matmul(psl[:, 0:tw], wvr8lo, rhs8,
                                     start=False, stop=True, perf_mode=DR)
                    psh0 = pstile()
                    psh = psh0[0:64, 0:tw]
                    nc.tensor.matmul(psh, wv8hi, rhs8,
                                     start=True, stop=False, perf_mode=DR)
                    nc.tensor.matmul(psh, wv8hi, rhsr,
                                     start=False, stop=False, perf_mode=DR)
                    nc.tensor.matmul(psh, wvr8hi, rhs8,
                                     start=False, stop=True, perf_mode=DR)
                    zAv = zA[:, :].rearrange("p (r w) -> p r w", w=PW)
                    rAv = rA[:, :].rearrange("p (r w) -> p r w", w=PW)
                    zBv = zB[:, :].rearrange("p (r w) -> p r w", w=PW)
                    zw = zAv[:, slot0 + row0: slot0 + row0 + nr, IMG0:IMG0 + W]
                    rw = rAv[:, slot0 + row0: slot0 + row0 + nr, IMG0:IMG0 + W]
                    bzw = zBv[0:64, slot0 + row0: slot0 + row0 + nr, IMG0:IMG0 + W]
                    brw = zBv[64:128, slot0 + row0: slot0 + row0 + nr, IMG0:IMG0 + W]
                    pslw = psl[:, 0:tw].rearrange("p (r w) -> p r w", w=W)
                    pshw = psh.rearrange("p (r w) -> p r w", w=W)
                    nc.scalar.copy(zw, pslw)
                    nc.vector.scalar_tensor_tensor(rw, zw, -1.0, pslw, A.mult, A.add)
                    nc.scalar.copy(bzw, pshw)
                    nc.vector.scalar_tensor_tensor(brw, bzw, -1.0, pshw,
                                                   A.mult, A.add)
            if c == 3:
                xts.pop(s)

        def emit_dwqk_chunk(s, c):
            g = c
            if c == 0:
                qbs[s] = [qpool.tile([128, 2 * 4 * W], bf16, tag=f"qb{b}",
                                     name=f"qb{b}") for b in range(4)]
                qkis[s] = [kpool.tile([128, SLAB * W], bf16, tag=f"qki{b}",
                                      name=f"qki{b}") for b in range(4)]
            z8, qb, qki = z8s[s], qbs[s], qkis[s]
            for b in range(4):
              with nc.named_scope(f"dwqk{s}"):
                zp = z8[b][:, :].ap[0][0]
                pd = pstile()
                for p in range(5):
                    dy0, dx0 = TAPS[2 * p]
                    o0 = (4 * g + 1 + dy0) * PW + IMG0 + dx0
                    if 2 * p + 1 < 9:
                        dy1, dx1 = TAPS[2 * p + 1]
                        o1 = (4 * g + 1 + dy1) * PW + IMG0 + dx1
                    else:
                        o1 = o0 + 2
                    rhs = ap_dims(z8[b][:, :],
                                  [[zp, 128], [o1 - o0, 2], [PW, 4], [1, W]],
                                  extra_offset=o0)
                    nc.tensor.matmul(pd[:, :], dwq8v[:, b, p, :, :], rhs,
                                     start=(p == 0), stop=(p == 4),
                                     perf_mode=DR)
                cpy(qb[b][:, (g % 2) * 512:(g % 2) * 512 + 512], pd[:, :])
                if g % 2 == 1:
                    h = g // 2
                    qkv_view = qki[b][:, h * 8 * W:(h + 1) * 8 * W].rearrange(
                        "p (t s) -> p t s", t=8)
                    nc.sync.dma_start_transpose(qkv_view, qb[b][:, :])
            if c == 3:
                z8s.pop(s)

        def emit_dwv_chunk(s, c):
            r0 = SLAB * s
            g = c
            zA, rA, zB = zv8s[s]
            with nc.named_scope(f"dwv{s}"):
                zp = zA[:, :].ap[0][0]
                rp = rA[:, :].ap[0][0]
                bp = zB[:, :].ap[0][0]
                psV = pstile()
                psV20 = pstile()
                psV2 = psV20[0:64, 0:512]
                # A: 9 dual-weight taps (w8 in j0, wr in j1, j-stride 0)
                for t in range(9):
                    dy, dx = TAPS[t]
                    ot = (4 * g + 1 + dy) * PW + IMG0 + dx
                    rhs = ap_dims(zA[:, :],
                                  [[zp, 128], [0, 2], [PW, 4], [1, W]],
                                  extra_offset=ot)
                    nc.tensor.matmul(psV[:, :], dwvA8v[:, t, :, :], rhs,
                                     start=(t == 0), stop=False, perf_mode=DR)
                # rA: 5 tap-paired w8 instrs, accumulate onto psV
                for p in range(5):
                    dy0, dx0 = TAPS[2 * p]
                    o0 = (4 * g + 1 + dy0) * PW + IMG0 + dx0
                    if 2 * p + 1 < 9:
                        dy1, dx1 = TAPS[2 * p + 1]
                        o1 = (4 * g + 1 + dy1) * PW + IMG0 + dx1
                    else:
                        o1 = o0 + 2
                    rhs = ap_dims(rA[:, :],
                                  [[rp, 128], [o1 - o0, 2], [PW, 4], [1, W]],
                                  extra_offset=o0)
                    nc.tensor.matmul(psV[:, :], dwvrA8v[:, p, :, :], rhs,
                                     start=False, stop=(p == 4), perf_mode=DR)
                # B: 9 taps; rows 0-63 dual-weight z8v hi, rows 64-127 r hi (w8)
                for t in range(9):
                    dy, dx = TAPS[t]
                    ot = (4 * g + 1 + dy) * PW + IMG0 + dx
                    rhs = ap_dims(zB[:, :],
                                  [[bp, 128], [0, 2], [PW, 4], [1, W]],
                                  extra_offset=ot)
                    nc.tensor.matmul(psV2, dwvB8v[:, t, :, :], rhs,
                                     start=(t == 0), stop=(t == 8), perf_mode=DR)
                # evac into vout8/voutr8 (j-split 128+64): ch0-127 | ch128-191
                c0 = (r0 + 4 * g) * W
                nc.scalar.copy(vout8[0:128, c0:c0 + 512], psV[:, :])
                nc.vector.scalar_tensor_tensor(
                    voutr8[0:128, c0:c0 + 512], vout8[0:128, c0:c0 + 512],
                    -1.0, psV[:, :], A.mult, A.add)
                nc.scalar.copy(vout8[0:64, HW + c0:HW + c0 + 512], psV2)
                nc.vector.scalar_tensor_tensor(
                    voutr8[0:64, HW + c0:HW + c0 + 512],
                    vout8[0:64, HW + c0:HW + c0 + 512],
                    -1.0, psV2, A.mult, A.add)
            if c == 3:
                zv8s.pop(s)

        def emit_gram_chunk(s, c):
            qki = qkis[s]
            for u in range(4 * c, 4 * c + 4):
                g_idx = SLAB * s + u
                for b in range(4):
                  with nc.named_scope(f"gram{s}"):
                    lhsT = qki[b][:, u * W:(u + 1) * W]
                    rhs = ap_dims(qki[b][:, :],
                                  [[qki[b][:, :].ap[0][0], 128], [32, 4], [1, 24]],
                                  extra_offset=u * W)
                    nc.tensor.matmul(gram[:, 96 * b:96 * (b + 1)], lhsT, rhs,
                                     start=(g_idx == 0), stop=(g_idx == H - 1),
                                     skip_group_check=True)
            if c == 3:
                qbs.pop(s), qkis.pop(s)

        # ---------- pipeline (flat chunk stream with per-stage lags) ----------
        NCHUNK = 4 * NSLABS
        LAGD, LAGG = 2, 5
        emit_loads(0)
        for pos in range(NCHUNK + LAGG + 1):
            s, c = divmod(pos, 4)
            if c == 0 and 0 < s + 1 < NSLABS + 1 and s + 1 < NSLABS:
                emit_loads(s + 1)
            if pos < NCHUNK:
                emit_qkv_chunk(s, c)
            p = pos - LAGD
            if 0 <= p < NCHUNK:
                s2, c2 = divmod(p, 4)
                emit_dwqk_chunk(s2, c2)
                if s2 < NSLABS - 1:
                    emit_dwv_chunk(s2, c2)
            p = pos - LAGG
            if 0 <= p < NCHUNK:
                s3, c3 = divmod(p, 4)
                emit_gram_chunk(s3, c3)
                if s3 == NSLABS - 1 and c3 in (2, 3):
                    emit_dwv_chunk(NSLABS - 1, c3 - 2)

        # ---------- norms ----------
        gm = mpool.tile([128, 4 * 96], f32, tag="gm", bufs=1)
        nc.vector.tensor_tensor(gm[:, :], gram[:, :], msk[:, :], A.mult)
        s_sb = mpool.tile([128, 4], f32, tag="ssb")
        nc.vector.tensor_reduce(s_sb[:, :],
                                gm[:, :].rearrange("p (g c) -> p g c", g=4),
                                AX.X, A.add)
        ns = mpool.tile([128, 4], f32, tag="ns")
        nc.scalar.sqrt(ns[:, :], s_sb[:, :])
        nsc = mpool.tile([128, 4], f32, tag="nsc")
        nc.vector.tensor_scalar_max(nsc[:, :], ns[:, :], EPS)
        ry = mpool.tile([128, 4], f32, tag="ry")
        nc.vector.reciprocal(ry[:, :], nsc[:, :])
        t1 = mpool.tile([128, 4], f32, tag="t1")
        nc.vector.tensor_tensor(t1[:, :], s_sb[:, :], ry[:, :], A.mult)
        t2 = mpool.tile([128, 4], f32, tag="t2")
        nc.vector.tensor_add(t2[:, :], nsc[:, :], t1[:, :])
        ns2 = mpool.tile([128, 4], f32, tag="ns2")
        nc.vector.tensor_scalar_mul(ns2[:, :], t2[:, :], 0.5)
        ns3 = mpool.tile([128, 4], f32, tag="ns3")
        nc.vector.tensor_scalar_max(ns3[:, :], ns2[:, :], EPS)
        rn = mpool.tile([128, 4], f32, tag="rn")
        nc.vector.reciprocal(rn[:, :], ns3[:, :])
        rkt = mpool.tile([128, 4], f32, tag="rkt")
        nc.vector.tensor_tensor(rkt[:, :], rn[:, :], tmap[:, :], A.mult)
        rq = mpool.tile([24, 8], f32, tag="rq")
        nc.sync.dma_start(rq[0:24, 1:8:2], rn[64:88, 0:4])

        # ---------- softmax + A blockdiag ----------
        a0 = apool.tile([96, C], f32, tag="a0")
        a1 = apool.tile([96, C], f32, tag="a1")
        nc.vector.memset(a0[:, :], 0.0)
        nc.vector.memset(a1[:, :], 0.0)
        bt = mpool.tile([128, 8 * CD], f32, tag="bt", bufs=1)
        AF_ = AF
        for h in range(NUM_HEADS):
            b = h // 2
            kbase = 32 if h % 2 == 0 else 96
            qcol = 0 if h % 2 == 0 else 48
            nc.vector.tensor_scalar_mul(
                bt[kbase:kbase + CD, CD * h:CD * (h + 1)],
                gram[kbase:kbase + CD, 96 * b + qcol:96 * b + qcol + CD],
                rkt[kbase:kbase + CD, b:b + 1])
            ptr0 = pstile()
            ptr = ptr0[0:CD, 0:CD]
            nc.tensor.transpose(ptr,
                                bt[kbase:kbase + CD, CD * h:CD * (h + 1)],
                                eye[kbase:kbase + CD, 0:CD],
                                tile_position=(kbase, 0))
            es = mpool.tile([CD, CD], f32, tag="es")
            se = mpool.tile([CD, 1], f32, tag="se")
            rqh = (rn[0:24, b:b + 1] if h % 2 == 0
                   else rq[0:24, h:h + 1])
            nc.scalar.activation(es[:, :], ptr, AF_.Exp,
                                 bias=0.0, scale=rqh,
                                 accum_out=se[:, :])
            rse = mpool.tile([CD, 1], f32, tag="rse")
            nc.vector.reciprocal(rse[:, :], se[:, :])
            ah = mpool.tile([CD, CD], f32, tag="ah")
            nc.vector.tensor_scalar_mul(ah[:, :], es[:, :], rse[0:CD, 0:1])
            adst = a0 if h < 4 else a1
            r0 = 24 * (h % 4)
            nc.sync.dma_start(adst[r0:r0 + CD, CD * h:CD * (h + 1)], ah[:, :])

        for c in (2, 3):
            emit_dwv_chunk(NSLABS - 1, c)

        # ---------- M^T = A_bd^T @ (8*W_proj^T) (fp32), then fp8 + residual ----
        # wpT is pre-scaled x8 host-side so M8/Mr stay clear of the fp8
        # subnormal floor; the x8 is undone at the out-evac (scale 1/8).
        # mt8 rows = v-ch: j0 rows 0-127 = ch0-127, j1 rows 0-63 = ch128-191,
        # j1 rows 64-127 zero (matches vout8 pad).
        mt8 = cpool.tile([128, 2 * C], f8, tag="mt8")
        mtr8 = cpool.tile([128, 2 * C], f8, tag="mtr8")
        nc.gpsimd.memset(mt8[64:128, C:2 * C], 0.0)
        nc.gpsimd.memset(mtr8[64:128, C:2 * C], 0.0)
        pmtA0 = pstile()
        pmtA = pmtA0[0:128, 0:C]
        nc.tensor.matmul(pmtA, a0[:, 0:128], wp0[:, :], start=True, stop=False)
        nc.tensor.matmul(pmtA, a1[:, 0:128], wp1[:, :], start=False, stop=True)
        nc.scalar.copy(mt8[0:128, 0:C], pmtA)
        nc.vector.scalar_tensor_tensor(mtr8[0:128, 0:C], mt8[0:128, 0:C], -1.0,
                                       pmtA, A.mult, A.add)
        pmtB0 = pstile()
        pmtB = pmtB0[0:64, 0:C]
        nc.tensor.matmul(pmtB, a0[:, 128:192], wp0[:, :], start=True, stop=False)
        nc.tensor.matmul(pmtB, a1[:, 128:192], wp1[:, :], start=False, stop=True)
        nc.scalar.copy(mt8[0:64, C:2 * C], pmtB)
        nc.vector.scalar_tensor_tensor(mtr8[0:64, C:2 * C], mt8[0:64, C:2 * C],
                                       -1.0, pmtB, A.mult, A.add)
        mt8v = mt8[:, :].rearrange("p (j m) -> p j m", j=2)
        mtr8v = mtr8[:, :].rearrange("p (j m) -> p j m", j=2)

        # ---------- out = M @ v (fp8 DR: M8 v8 + Mr v8 + M8 vr, scaled 1/8) ----
        CHUNK = 1024
        vp = vout8[:, :].ap[0][0]
        vrp = voutr8[:, :].ap[0][0]
        _oe = [0]

        def cpy_scaled(dst, src):
            _oe[0] += 1
            if _oe[0] % 2 == 0:
                nc.scalar.activation(dst, src, AF.Copy, bias=0.0, scale=0.125)
            else:
                nc.vector.tensor_scalar_mul(dst, src, 0.125)

        oa = ob = None
        for t0 in range(0, HW, 512):
          with nc.named_scope("mv"):
            if t0 % CHUNK == 0:
                oa = opool.tile([128, CHUNK], f16, tag="oa")
                ob = opool.tile([64, CHUNK], f16, tag="ob")
            c0 = t0 % CHUNK
            rhs8 = ap_dims(vout8[:, :], [[vp, 128], [HW, 2], [1, 512]],
                           extra_offset=t0)
            rhsr = ap_dims(voutr8[:, :], [[vrp, 128], [HW, 2], [1, 512]],
                           extra_offset=t0)
            pa = pstile()
            nc.tensor.matmul(pa[:, :], mt8v[:, :, 0:128], rhs8,
                             start=True, stop=False, perf_mode=DR)
            nc.tensor.matmul(pa[:, :], mtr8v[:, :, 0:128], rhs8,
                             start=False, stop=False, perf_mode=DR)
            nc.tensor.matmul(pa[:, :], mt8v[:, :, 0:128], rhsr,
                             start=False, stop=True, perf_mode=DR)
            cpy_scaled(oa[:, c0:c0 + 512], pa[:, :])
            pb0 = pstile()
            pb = pb0[0:64, 0:512]
            nc.tensor.matmul(pb, mt8v[:, :, 128:192], rhs8,
                             start=True, stop=False, perf_mode=DR)
            nc.tensor.matmul(pb, mtr8v[:, :, 128:192], rhs8,
                             start=False, stop=False, perf_mode=DR)
            nc.tensor.matmul(pb, mt8v[:, :, 128:192], rhsr,
                             start=False, stop=True, perf_mode=DR)
            cpy_scaled(ob[:, c0:c0 + 512], pb)
            if t0 % CHUNK == CHUNK - 512:
                b0 = t0 + 512 - CHUNK
                nc.sync.dma_start(outd[0:128, b0:b0 + CHUNK], oa[:, :])
                nc.gpsimd.dma_start(outd[128:192, b0:b0 + CHUNK], ob[:, :])

    nc.compile()
    return nc


def _host_consts(w_qkv, w_dw, w_proj, temperature):
    import ml_dtypes
    f8 = ml_dtypes.float8_e4m3

    wq = np.asarray(w_qkv, np.float32)            # [576, 192]
    wd = np.asarray(w_dw, np.float32).reshape(3 * C, 3, 3)
    wpT = np.ascontiguousarray(np.asarray(w_proj, np.float32).T)

    # tap index -> (dy, dx)
    # wq8 [128, 4, 2, 128]
    wq8 = np.zeros((128, 4, 2, 128), np.float32)
    dwq8 = np.zeros((128, 4, 5, 2, 128), np.float32)
    for b in range(4):
        for sb, chb in _slot_groups(b):
            for i in range(CD):
                ch = chb + i
                s = sb + i
                wq8[0:128, b, 0, s] = wq[ch, 0:128]
                wq8[64:128, b, 1, s] = wq[ch, 128:192]
                for t, (dy, dx) in enumerate(TAPS):
                    dwq8[s, b, t // 2, t % 2, s] = wd[ch, dy + 1, dx + 1]

    # v 1x1 weights: out-blocks [128]+[64] packed [128, 2*128 + 2*64]
    wv = np.zeros((128, 2 * 128 + 2 * 64), np.float32)
    for c in range(128):
        ch = 384 + c
        wv[0:128, c] = wq[ch, 0:128]
        wv[64:128, 128 + c] = wq[ch, 128:192]
    for c in range(64):
        ch = 384 + 128 + c
        wv[0:128, 256 + c] = wq[ch, 0:128]
        wv[64:128, 320 + c] = wq[ch, 128:192]
    wv8q = wv.astype(f8)
    wvr8 = (wv - wv8q.astype(np.float32)).astype(f8)

    # v dw weights in fp8 main + residual
    wdv = wd[384:576]                                 # [192, 3, 3]
    wdv_taps = np.stack([wdv[:, dy + 1, dx + 1] for (dy, dx) in TAPS],
                        axis=1)                       # [192, 9]
    w8v = wdv_taps.astype(f8).astype(np.float32)
    wrv = (wdv_taps - w8v).astype(f8).astype(np.float32)
    dwvA8 = np.zeros((128, 9, 2, 128), np.float32)
    dwvrA8 = np.zeros((128, 5, 2, 128), np.float32)
    dwvB8 = np.zeros((128, 9, 2, 64), np.float32)
    for s in range(128):
        for t in range(9):
            dwvA8[s, t, 0, s] = w8v[s, t]
            dwvA8[s, t, 1, s] = wrv[s, t]
        for p in range(5):
            for j in range(2):
                if 2 * p + j < 9:
                    dwvrA8[s, p, j, s] = w8v[s, 2 * p + j]
    for p in range(64):
        ch = 128 + p
        for t in range(9):
            dwvB8[p, t, 0, p] = w8v[ch, t]
            dwvB8[p, t, 1, p] = wrv[ch, t]
            dwvB8[64 + p, t, 0, p] = w8v[ch, t]

    gmask = np.zeros((128, 4 * 96), np.float32)
    for sb, cc in ((0, 0), (32, 24), (64, 48), (96, 72)):
        for i in range(CD):
            for b in range(4):
                gmask[sb + i, 96 * b + cc + i] = 1.0

    tmap = np.ones((128, 4), np.float32)
    tf = np.asarray(temperature, np.float32).reshape(-1)
    for b in range(4):
        tmap[32:56, b] = tf[2 * b]
        tmap[96:120, b] = tf[2 * b + 1]

    return dict(
        wq8=wq8.reshape(128, -1).astype(f8),
        dwq8=dwq8.reshape(128, -1).astype(f8),
        wv8=wv8q,
        wvr8=wvr8,
        dwvA8=dwvA8.reshape(128, -1).astype(f8),
        dwvrA8=dwvrA8.reshape(128, -1).astype(f8),
        dwvB8=dwvB8.reshape(128, -1).astype(f8),
        wpT=wpT * 8.0,
        gmask=gmask,
        tmap=tmap,
        eye=_eye_slim(),
    )


def _eye_slim():
    e = np.zeros((128, 24), np.float32)
    for kb in (32, 96):
        for i in range(24):
            e[kb + i, i] = 1.0
    return e


def kernel(x, w_qkv, w_dw, w_proj, temperature, _trace=False):
    import ml_dtypes
    from concourse.bass_utils import run_bass_kernel_spmd

    if "nc" not in _CACHE:
        _CACHE["nc"] = _build()
    nc = _CACHE["nc"]

    consts = _host_consts(w_qkv, w_dw, w_proj, temperature)
    xr = np.ascontiguousarray(np.asarray(x, np.float32).reshape(NCORES, C, HW))
    x8 = xr.astype(ml_dtypes.float8_e4m3)
    xr8 = (xr - x8.astype(np.float32)).astype(ml_dtypes.float8_e4m3)
    in_maps = []
    for bb in range(NCORES):
        m = {"x8": x8[bb], "xr8": xr8[bb]}
        m.update(consts)
        in_maps.append(m)

    try:
        br = run_bass_kernel_spmd(nc, in_maps, core_ids=list(range(NCORES)),
                                  trace=_trace)
    except ModuleNotFoundError:
        br = run_bass_kernel_spmd(nc, in_maps, core_ids=list(range(NCORES)),
                                  trace=False)
    out = np.stack([np.asarray(r["out"], dtype=np.float32) for r in br.results],
                   axis=0).reshape(NCORES, C, H, W)
    _CACHE["last_results"] = br
    return out



# revision 19
# speedup vs baseline: 1.0515x; 1.0515x over previous
"""Trainium2 Bass kernel for Restormer-style transposed (channel) attention, v2.

Per-core (1 of 8 batch elements), built around the TimelineSim cost model
(matmul cost = output free-size; fp8e4m3 DoubleRow = 0.5 cyc/col):

  q/k path (errors wash out through the softmax normalization):
    z_qk = Wqk8 @ x8            fp8 DoubleRow, 192-contraction in 1 instr
    dwconv 3x3                  5 DR diag tap-pair matmuls per block (2.5 cyc/px)
    -> bf16 qb -> xbar DMA transpose -> [px, slot] qki tiles
    gram G += qki^T qki         bf16, compact-col strided rhs
  v path (kept accurate):
    z_v = Wv @ x16              bf16
    dwconv = DR(fp8(z_v)) + DR(fp8(z_v - fp8(z_v)))   exact to ~0.2%
    vout fp16 resident in SBUF
  tail: norms from gram diag, softmax per head, M^T = A_bd^T Wproj^T,
        out = M @ vout streamed to HBM.

Slot layout (32-aligned, 4 blocks of 128):
  block b: [q_{2b} 0:24 | pad | k_{2b} 32:56 | pad | q_{2b+1} 64:88 | pad |
            k_{2b+1} 96:120 | pad]
"""
import numpy as np

NUM_HEADS = 8
C = 192
H = W = 128
HW = H * W
CD = 24
NCORES = 8
SLAB = 16
NSLABS = H // SLAB
EPS = 1e-12
PW = W + 4
IMG0 = 2
NR = SLAB + 2          # z8 slab rows incl halo

# tap order chosen so DR pairs have EVEN offset deltas (hw requirement):
# pairs: ((-1,-1),(-1,1)) ((0,-1),(0,1)) ((1,-1),(1,1)) ((-1,0),(0,0)) ((1,0),zero)
TAPS = [(-1, -1), (-1, 1), (0, -1), (0, 1), (1, -1), (1, 1), (-1, 0), (0, 0), (1, 0)]

# per-block slot groups: (slot_base, qkv_ch_base)
def _slot_groups(b):
    return [(0, 48 * b), (32, 192 + 48 * b), (64, 48 * b + 24), (96, 192 + 48 * b + 24)]


_CACHE = {}


def _stripes(ncols):
    """split ncols into row-aligned stripes of >=256 (multiples of 128)."""
    out = []
    rem = ncols
    while rem > 0:
        t = min(512, rem)
        if rem - t == 128:
            t = 384
        out.append(t)
        rem -= t
    return out


def _build():
    import concourse.bass as bass
    import concourse.mybir as mybir
    import concourse.tile as tile
    from concourse import bacc
    from contextlib import ExitStack
    import bass_rust

    dt = mybir.dt
    A = mybir.AluOpType
    AF = mybir.ActivationFunctionType
    AX = mybir.AxisListType
    DR = mybir.MatmulPerfMode.DoubleRow
    f32, bf16, f16, f8, f32r = dt.float32, dt.bfloat16, dt.float16, dt.float8e4, dt.float32r

    def ap_dims(ap, dims, extra_offset=0):
        c = ap.copy()
        c.ap = bass_rust.VecI64Pair(dims)
        c.offset = ap.offset + extra_offset
        return c

    nc = bacc.Bacc("TRN2", num_devices=NCORES)

    x8d = nc.dram_tensor("x8", [C, HW], f8, kind="ExternalInput").ap()
    xr8d = nc.dram_tensor("xr8", [C, HW], f8, kind="ExternalInput").ap()
    wq8d = nc.dram_tensor("wq8", [128, 4 * 2 * 128], f8, kind="ExternalInput").ap()
    dwq8d = nc.dram_tensor("dwq8", [128, 4 * 5 * 2 * 128], f8, kind="ExternalInput").ap()
    # v 1x1 weights, out-blocks [128]+[64]: [.., 0:256]=lo j2 m128, [.., 256:384]=hi j2 m64
    wv8d = nc.dram_tensor("wv8", [128, 2 * 128 + 2 * 64], f8, kind="ExternalInput").ap()
    wvr8d = nc.dram_tensor("wvr8", [128, 2 * 128 + 2 * 64], f8, kind="ExternalInput").ap()
    # v dw: A = dual-weight (w8,wr) per tap for ch0-127; rA = tap-paired w8 for r ch0-127;
    # B = mixed block (rows 0-63: z8v ch128-191 dual; rows 64-127: r ch128-191 single w8)
    dwvA8d = nc.dram_tensor("dwvA8", [128, 9 * 2 * 128], f8, kind="ExternalInput").ap()
    dwvrA8d = nc.dram_tensor("dwvrA8", [128, 5 * 2 * 128], f8, kind="ExternalInput").ap()
    dwvB8d = nc.dram_tensor("dwvB8", [128, 9 * 2 * 64], f8, kind="ExternalInput").ap()
    zpadd = nc.dram_tensor("zpad", [64, HW], f8, kind="ExternalInput").ap()
    wpTd = nc.dram_tensor("wpT", [C, C], f32, kind="ExternalInput").ap()
    mskd = nc.dram_tensor("gmask", [128, 4 * 96], f32, kind="ExternalInput").ap()
    tmpd = nc.dram_tensor("tmap", [128, 4], f32, kind="ExternalInput").ap()
    eyed = nc.dram_tensor("eye", [128, 24], f32, kind="ExternalInput").ap()
    outd = nc.dram_tensor("out", [C, HW], f16, kind="ExternalOutput").ap()

    with tile.TileContext(nc) as tc:
      with ExitStack() as _es:
        cpool = _es.enter_context(tc.tile_pool(name="const", bufs=1))
        xpool = _es.enter_context(tc.tile_pool(name="xin", bufs=2))
        zpool = _es.enter_context(tc.tile_pool(name="zst", bufs=2))
        qpool = _es.enter_context(tc.tile_pool(name="qbt", bufs=2))
        kpool = _es.enter_context(tc.tile_pool(name="qki", bufs=2))
        vpool = _es.enter_context(tc.tile_pool(name="vout", bufs=1))
        mpool = _es.enter_context(tc.tile_pool(name="sm", bufs=2))
        apool = _es.enter_context(tc.tile_pool(name="abd", bufs=1))
        opool = _es.enter_context(tc.tile_pool(name="outs", bufs=3))
        psP = _es.enter_context(tc.tile_pool(name="psP", bufs=7, space="PSUM"))
        psG = _es.enter_context(tc.tile_pool(name="psG", bufs=1, space="PSUM"))

        def pstile():
            return psP.tile([128, 512], f32, tag="ps", name="ps")

        # ---------- constants (ACT dma queue; x streams on sync) ----------
        wq8 = cpool.tile([128, 4 * 2 * 128], f8, tag="wq8")
        nc.scalar.dma_start(wq8[:, :], wq8d[:, :])
        wv8 = cpool.tile([128, 2 * 128 + 2 * 64], f8, tag="wv8")
        nc.scalar.dma_start(wv8[:, :], wv8d[:, :])
        wvr8 = cpool.tile([128, 2 * 128 + 2 * 64], f8, tag="wvr8")
        nc.scalar.dma_start(wvr8[:, :], wvr8d[:, :])
        dwq8 = cpool.tile([128, 4 * 5 * 2 * 128], f8, tag="dwq8")
        nc.scalar.dma_start(dwq8[:, :], dwq8d[:, :])
        dwvA8 = cpool.tile([128, 9 * 2 * 128], f8, tag="dwvA8")
        nc.scalar.dma_start(dwvA8[:, :], dwvA8d[:, :])
        dwvrA8 = cpool.tile([128, 5 * 2 * 128], f8, tag="dwvrA8")
        nc.scalar.dma_start(dwvrA8[:, :], dwvrA8d[:, :])
        dwvB8 = cpool.tile([128, 9 * 2 * 64], f8, tag="dwvB8")
        nc.scalar.dma_start(dwvB8[:, :], dwvB8d[:, :])
        msk = cpool.tile([128, 4 * 96], f32, tag="msk")
        nc.scalar.dma_start(msk[:, :], mskd[:, :])
        tmap = cpool.tile([128, 4], f32, tag="tmap")
        nc.scalar.dma_start(tmap[:, :], tmpd[:, :])
        eye = cpool.tile([128, 24], f32, tag="eye")
        nc.scalar.dma_start(eye[:, :], eyed[:, :])
        wp0 = cpool.tile([96, C], f32, tag="wp0")
        nc.scalar.dma_start(wp0[:, :], wpTd[0:96, :])
        wp1 = cpool.tile([96, C], f32, tag="wp1")
        nc.scalar.dma_start(wp1[:, :], wpTd[96:192, :])

        wq8v = wq8[:, :].rearrange("p (b j s) -> p b j s", b=4, j=2)
        dwq8v = dwq8[:, :].rearrange("p (b k j s) -> p b k j s", b=4, k=5, j=2)
        wv8lo = wv8[:, 0:256].rearrange("p (j s) -> p j s", j=2)
        wv8hi = wv8[:, 256:384].rearrange("p (j s) -> p j s", j=2)
        wvr8lo = wvr8[:, 0:256].rearrange("p (j s) -> p j s", j=2)
        wvr8hi = wvr8[:, 256:384].rearrange("p (j s) -> p j s", j=2)
        dwvA8v = dwvA8[:, :].rearrange("p (t j s) -> p t j s", t=9, j=2)
        dwvrA8v = dwvrA8[:, :].rearrange("p (t j s) -> p t j s", t=5, j=2)
        dwvB8v = dwvB8[:, :].rearrange("p (t j s) -> p t j s", t=9, j=2)

        # vout in fp8 main+residual; contraction j-split 128+64 for M@v DR
        # (j1 rows 64-127 are zero pad, memset once on Pool)
        vout8 = vpool.tile([128, 2 * HW], f8, tag="vout8", name="vout8")
        voutr8 = vpool.tile([128, 2 * HW], f8, tag="voutr8", name="voutr8")
        nc.scalar.dma_start(vout8[64:128, HW:2 * HW], zpadd[:, :])
        nc.scalar.dma_start(voutr8[64:128, HW:2 * HW], zpadd[:, :])
        gram = psG.tile([128, 4 * 96], f32, tag="g")

        # evac engine round-robin (ACT / DVE alternating)
        _ev = [0]
        def cpy(dst, src):
            _ev[0] += 1
            if _ev[0] % 2 == 0:
                nc.scalar.copy(dst, src)
            else:
                nc.vector.tensor_copy(dst, src)

        # slab state carried across pipeline iterations
        z8s, zv8s, qbs, qkis, xts = {}, {}, {}, {}, {}

        def _slabmeta(s):
            r0 = SLAB * s
            lo, hi = max(0, r0 - 1), min(H - 1, r0 + SLAB)
            nrows = hi - lo + 1
            return r0, lo, nrows, lo - (r0 - 1)

        def emit_loads(s):
            r0, lo, nrows, slot0 = _slabmeta(s)
            ncols = nrows * W
            col0 = lo * W
            xs8 = xpool.tile([128, 2 * ncols], f8, tag="xs8")
            nc.sync.dma_start(xs8[:, 0:ncols], x8d[0:128, col0:col0 + ncols])
            # k-tile 1 holds x channels 64..191 (rows 0-63 have zero weights)
            nc.sync.dma_start(xs8[:, ncols:2 * ncols],
                              x8d[64:192, col0:col0 + ncols])
            xsr8 = xpool.tile([128, 2 * ncols], f8, tag="xsr8")
            nc.sync.dma_start(xsr8[:, 0:ncols], xr8d[0:128, col0:col0 + ncols])
            nc.sync.dma_start(xsr8[:, ncols:2 * ncols],
                              xr8d[64:192, col0:col0 + ncols])
            xts[s] = (xs8, xsr8, ncols, slot0)

        def emit_qkv_chunk(s, c):
            r0, lo, nrows, slot0 = _slabmeta(s)
            ncols = nrows * W
            if c == 0:
                z8 = [zpool.tile([128, NR * PW], f8, tag=f"z8_{b}", name=f"z8_{b}")
                      for b in range(4)]
                # zA: z8v ch0-127; rA: r ch0-127; B: rows 0-63 z8v ch128-191,
                # rows 64-127 r ch128-191
                zv8 = [zpool.tile([128, NR * PW], f8, tag=f"zv8_{v}", name=f"zv8_{v}")
                       for v in range(3)]
                if s < 2:
                    for t in z8 + zv8:
                        tv = t[:, :].rearrange("p (r w) -> p r w", w=PW)
                        nc.gpsimd.memset(tv[:, :, 0:IMG0], 0.0)
                        nc.gpsimd.memset(tv[:, :, IMG0 + W:PW], 0.0)
                if s == 0:
                    for t in z8 + zv8:
                        nc.gpsimd.memset(t[:, 0:PW], 0.0)
                if s == NSLABS - 1:
                    for t in z8 + zv8:
                        nc.gpsimd.memset(t[:, (NR - 1) * PW:NR * PW], 0.0)
                z8s[s], zv8s[s] = z8, zv8
            xs8, xsr8, ncols, slot0 = xts[s]
            z8, zv8 = z8s[s], zv8s[s]
            zA, rA, zB = zv8
            xs8p = xs8[:, :].ap[0][0]
            xsr8p = xsr8[:, :].ap[0][0]
            strs = _stripes(ncols)
            todo = [c] if c < 3 else [3] + list(range(4, len(strs)))
            for si in todo:
                tw = strs[si]
                t0 = sum(strs[:si])
                row0 = t0 // W
                nr = tw // W
                for b in range(4):
                  with nc.named_scope(f"qkvqk{s}"):
                    ps = pstile()
                    rhs = ap_dims(xs8[:, :], [[xs8p, 128], [ncols, 2], [1, tw]],
                                  extra_offset=t0)
                    nc.tensor.matmul(ps[0:128, 0:tw], wq8v[:, b, :, :], rhs,
                                     start=True, stop=True, perf_mode=DR)
                    zview = z8[b][:, :].rearrange("p (r w) -> p r w", w=PW)
                    cpy(zview[:, slot0 + row0: slot0 + row0 + nr, IMG0:IMG0 + W],
                        ps[0:128, 0:tw].rearrange("p (r w) -> p r w", w=W))
                with nc.named_scope(f"qkvv{s}"):
                    rhs8 = ap_dims(xs8[:, :], [[xs8p, 128], [ncols, 2], [1, tw]],
                                   extra_offset=t0)
                    rhsr = ap_dims(xsr8[:, :], [[xsr8p, 128], [ncols, 2], [1, tw]],
                                   extra_offset=t0)
                    psl = pstile()
                    nc.tensor.matmul(psl[:, 0:tw], wv8lo, rhs8,
                                     start=True, stop=False, perf_mode=DR)
                    nc.tensor.matmul(psl[:, 0:tw], wv8lo, rhsr,
                                     start=False, stop=False, perf_mode=DR)
                    nc.tensor.matmul(psl[:, 0:tw], wvr8lo, rhs8,
                                     start=False, stop=True, perf_mode=DR)
                    psh0 = pstile()
                    psh = psh0[0:64, 0:tw]
                    nc.tensor.matmul(psh, wv8hi, rhs8,
                                     start=True, stop=False, perf_mode=DR)
                    nc.tensor.matmul(psh, wv8hi, rhsr,
                                     start=False, stop=False, perf_mode=DR)
                    nc.tensor.matmul(psh, wvr8hi, rhs8,
                                     start=False, stop=True, perf_mode=DR)
                    zAv = zA[:, :].rearrange("p (r w) -> p r w", w=PW)
                    rAv = rA[:, :].rearrange("p (r w) -> p r w", w=PW)
                    zBv = zB[:, :].rearrange("p (r w) -> p r w", w=PW)
                    zw = zAv[:, slot0 + row0: slot0 + row0 + nr, IMG0:IMG0 + W]
                    rw = rAv[:, slot0 + row0: slot0 + row0 + nr, IMG0:IMG0 + W]
                    bzw = zBv[0:64, slot0 + row0: slot0 + row0 + nr, IMG0:IMG0 + W]
                    brw = zBv[64:128, slot0 + row0: slot0 + row0 + nr, IMG0:IMG0 + W]
                    pslw = psl[:, 0:tw].rearrange("p (r w) -> p r w", w=W)
                    pshw = psh.rearrange("p (r w) -> p r w", w=W)
                    nc.scalar.copy(zw, pslw)
                    nc.vector.scalar_tensor_tensor(rw, zw, -1.0, pslw, A.mult, A.add)
                    nc.scalar.copy(bzw, pshw)
                    nc.vector.scalar_tensor_tensor(brw, bzw, -1.0, pshw,
                                                   A.mult, A.add)
            if c == 3:
                xts.pop(s)

        def emit_dwqk_chunk(s, c):
            g = c
            if c == 0:
                qbs[s] = [qpool.tile([128, 2 * 4 * W], bf16, tag=f"qb{b}",
                                     name=f"qb{b}") for b in range(4)]
                qkis[s] = [kpool.tile([128, SLAB * W], bf16, tag=f"qki{b}",
                                      name=f"qki{b}") for b in range(4)]
            z8, qb, qki = z8s[s], qbs[s], qkis[s]
            for b in range(4):
              with nc.named_scope(f"dwqk{s}"):
                zp = z8[b][:, :].ap[0][0]
                pd = pstile()
                for p in range(5):
                    dy0, dx0 = TAPS[2 * p]
                    o0 = (4 * g + 1 + dy0) * PW + IMG0 + dx0
                    if 2 * p + 1 < 9:
                        dy1, dx1 = TAPS[2 * p + 1]
                        o1 = (4 * g + 1 + dy1) * PW + IMG0 + dx1
                    else:
                        o1 = o0 + 2
                    rhs = ap_dims(z8[b][:, :],
                                  [[zp, 128], [o1 - o0, 2], [PW, 4], [1, W]],
                                  extra_offset=o0)
                    nc.tensor.matmul(pd[:, :], dwq8v[:, b, p, :, :], rhs,
                                     start=(p == 0), stop=(p == 4),
                                     perf_mode=DR)
                cpy(qb[b][:, (g % 2) * 512:(g % 2) * 512 + 512], pd[:, :])
                if g % 2 == 1:
                    h = g // 2
                    qkv_view = qki[b][:, h * 8 * W:(h + 1) * 8 * W].rearrange(
                        "p (t s) -> p t s", t=8)
                    nc.sync.dma_start_transpose(qkv_view, qb[b][:, :])
            if c == 3:
                z8s.pop(s)

        def emit_dwv_chunk(s, c):
            r0 = SLAB * s
            g = c
            zA, rA, zB = zv8s[s]
            with nc.named_scope(f"dwv{s}"):
                zp = zA[:, :].ap[0][0]
                rp = rA[:, :].ap[0][0]
                bp = zB[:, :].ap[0][0]
                psV = pstile()
                psV20 = pstile()
                psV2 = psV20[0:64, 0:512]
                # A: 9 dual-weight taps (w8 in j0, wr in j1, j-stride 0)
                for t in range(9):
                    dy, dx = TAPS[t]
                    ot = (4 * g + 1 + dy) * PW + IMG0 + dx
                    rhs = ap_dims(zA[:, :],
                                  [[zp, 128], [0, 2], [PW, 4], [1, W]],
                                  extra_offset=ot)
                    nc.tensor.matmul(psV[:, :], dwvA8v[:, t, :, :], rhs,
                                     start=(t == 0), stop=False, perf_mode=DR)
                # rA: 5 tap-paired w8 instrs, accumulate onto psV
                for p in range(5):
                    dy0, dx0 = TAPS[2 * p]
                    o0 = (4 * g + 1 + dy0) * PW + IMG0 + dx0
                    if 2 * p + 1 < 9:
                        dy1, dx1 = TAPS[2 * p + 1]
                        o1 = (4 * g + 1 + dy1) * PW + IMG0 + dx1
                    else:
                        o1 = o0 + 2
                    rhs = ap_dims(rA[:, :],
                                  [[rp, 128], [o1 - o0, 2], [PW, 4], [1, W]],
                                  extra_offset=o0)
                    nc.tensor.matmul(psV[:, :], dwvrA8v[:, p, :, :], rhs,
                                     start=False, stop=(p == 4), perf_mode=DR)
                # B: 9 taps; rows 0-63 dual-weight z8v hi, rows 64-127 r hi (w8)
                for t in range(9):
                    dy, dx = TAPS[t]
                    ot = (4 * g + 1 + dy) * PW + IMG0 + dx
                    rhs = ap_dims(zB[:, :],
                                  [[bp, 128], [0, 2], [PW, 4], [1, W]],
                                  extra_offset=ot)
                    nc.tensor.matmul(psV2, dwvB8v[:, t, :, :], rhs,
                                     start=(t == 0), stop=(t == 8), perf_mode=DR)
                # evac into vout8/voutr8 (j-split 128+64): ch0-127 | ch128-191
                c0 = (r0 + 4 * g) * W
                nc.scalar.copy(vout8[0:128, c0:c0 + 512], psV[:, :])
                nc.vector.scalar_tensor_tensor(
                    voutr8[0:128, c0:c0 + 512], vout8[0:128, c0:c0 + 512],
                    -1.0, psV[:, :], A.mult, A.add)
                nc.scalar.copy(vout8[0:64, HW + c0:HW + c0 + 512], psV2)
                nc.vector.scalar_tensor_tensor(
                    voutr8[0:64, HW + c0:HW + c0 + 512],
                    vout8[0:64, HW + c0:HW + c0 + 512],
                    -1.0, psV2, A.mult, A.add)
            if c == 3:
                zv8s.pop(s)

        def emit_gram_chunk(s, c):
            qki = qkis[s]
            for u in range(4 * c, 4 * c + 4):
                g_idx = SLAB * s + u
                for b in range(4):
                  with nc.named_scope(f"gram{s}"):
                    lhsT = qki[b][:, u * W:(u + 1) * W]
                    rhs = ap_dims(qki[b][:, :],
                                  [[qki[b][:, :].ap[0][0], 128], [32, 4], [1, 24]],
                                  extra_offset=u * W)
                    nc.tensor.matmul(gram[:, 96 * b:96 * (b + 1)], lhsT, rhs,
                                     start=(g_idx == 0), stop=(g_idx == H - 1),
                                     skip_group_check=True)
            if c == 3:
                qbs.pop(s), qkis.pop(s)

        # ---------- pipeline (flat chunk stream with per-stage lags) ----------
        NCHUNK = 4 * NSLABS
        LAGD, LAGG = 2, 5
        emit_loads(0)
        for pos in range(NCHUNK + LAGG + 1):
            s, c = divmod(pos, 4)
            if c == 0 and 0 < s + 1 < NSLABS + 1 and s + 1 < NSLABS:
                emit_loads(s + 1)
            if pos < NCHUNK:
                emit_qkv_chunk(s, c)
            p = pos - LAGD
            if 0 <= p < NCHUNK:
                s2, c2 = divmod(p, 4)
                emit_dwqk_chunk(s2, c2)
                if s2 < NSLABS - 1:
                    emit_dwv_chunk(s2, c2)
            p = pos - LAGG
            if 0 <= p < NCHUNK:
                s3, c3 = divmod(p, 4)
                emit_gram_chunk(s3, c3)
                if s3 == NSLABS - 1 and c3 in (2, 3):
                    emit_dwv_chunk(NSLABS - 1, c3 - 2)

        # ---------- norms ----------
        gm = mpool.tile([128, 4 * 96], f32, tag="gm", bufs=1)
        nc.vector.tensor_tensor(gm[:, :], gram[:, :], msk[:, :], A.mult)
        s_sb = mpool.tile([128, 4], f32, tag="ssb")
        nc.vector.tensor_reduce(s_sb[:, :],
                                gm[:, :].rearrange("p (g c) -> p g c", g=4),
                                AX.X, A.add)
        ns = mpool.tile([128, 4], f32, tag="ns")
        nc.scalar.sqrt(ns[:, :], s_sb[:, :])
        nsc = mpool.tile([128, 4], f32, tag="nsc")
        nc.vector.tensor_scalar_max(nsc[:, :], ns[:, :], EPS)
        ry = mpool.tile([128, 4], f32, tag="ry")
        nc.vector.reciprocal(ry[:, :], nsc[:, :])
        t1 = mpool.tile([128, 4], f32, tag="t1")
        nc.vector.tensor_tensor(t1[:, :], s_sb[:, :], ry[:, :], A.mult)
        t2 = mpool.tile([128, 4], f32, tag="t2")
        nc.vector.tensor_add(t2[:, :], nsc[:, :], t1[:, :])
        ns2 = mpool.tile([128, 4], f32, tag="ns2")
        nc.vector.tensor_scalar_mul(ns2[:, :], t2[:, :], 0.5)
        ns3 = mpool.tile([128, 4], f32, tag="ns3")
        nc.vector.tensor_scalar_max(ns3[:, :], ns2[:, :], EPS)
        rn = mpool.tile([128, 4], f32, tag="rn")
        nc.vector.reciprocal(rn[:, :], ns3[:, :])
        rkt = mpool.tile([128, 4], f32, tag="rkt")
        nc.vector.tensor_tensor(rkt[:, :], rn[:, :], tmap[:, :], A.mult)
        rq = mpool.tile([24, 8], f32, tag="rq")
        nc.sync.dma_start(rq[0:24, 1:8:2], rn[64:88, 0:4])

        # ---------- softmax + A blockdiag ----------
        a0 = apool.tile([96, C], f32, tag="a0")
        a1 = apool.tile([96, C], f32, tag="a1")
        nc.vector.memset(a0[:, :], 0.0)
        nc.vector.memset(a1[:, :], 0.0)
        bt = mpool.tile([128, 8 * CD], f32, tag="bt", bufs=1)
        AF_ = AF
        for h in range(NUM_HEADS):
            b = h // 2
            kbase = 32 if h % 2 == 0 else 96
            qcol = 0 if h % 2 == 0 else 48
            nc.vector.tensor_scalar_mul(
                bt[kbase:kbase + CD, CD * h:CD * (h + 1)],
                gram[kbase:kbase + CD, 96 * b + qcol:96 * b + qcol + CD],
                rkt[kbase:kbase + CD, b:b + 1])
            ptr0 = pstile()
            ptr = ptr0[0:CD, 0:CD]
            nc.tensor.transpose(ptr,
                                bt[kbase:kbase + CD, CD * h:CD * (h + 1)],
                                eye[kbase:kbase + CD, 0:CD],
                                tile_position=(kbase, 0))
            es = mpool.tile([CD, CD], f32, tag="es")
            se = mpool.tile([CD, 1], f32, tag="se")
            rqh = (rn[0:24, b:b + 1] if h % 2 == 0
                   else rq[0:24, h:h + 1])
            nc.scalar.activation(es[:, :], ptr, AF_.Exp,
                                 bias=0.0, scale=rqh,
                                 accum_out=se[:, :])
            rse = mpool.tile([CD, 1], f32, tag="rse")
            nc.vector.reciprocal(rse[:, :], se[:, :])
            ah = mpool.tile([CD, CD], f32, tag="ah")
            nc.vector.tensor_scalar_mul(ah[:, :], es[:, :], rse[0:CD, 0:1])
            adst = a0 if h < 4 else a1
            r0 = 24 * (h % 4)
            nc.sync.dma_start(adst[r0:r0 + CD, CD * h:CD * (h + 1)], ah[:, :])

        for c in (2, 3):
            emit_dwv_chunk(NSLABS - 1, c)

        # ---------- M^T = A_bd^T @ (8*W_proj^T) (fp32), then fp8 + residual ----
        # wpT is pre-scaled x8 host-side so M8/Mr stay clear of the fp8
        # subnormal floor; the x8 is undone at the out-evac (scale 1/8).
        # mt8 rows = v-ch: j0 rows 0-127 = ch0-127, j1 rows 0-63 = ch128-191,
        # j1 rows 64-127 zero (matches vout8 pad).
        mt8 = cpool.tile([128, 2 * C], f8, tag="mt8")
        mtr8 = cpool.tile([128, 2 * C], f8, tag="mtr8")
        nc.gpsimd.memset(mt8[64:128, C:2 * C], 0.0)
        nc.gpsimd.memset(mtr8[64:128, C:2 * C], 0.0)
        pmtA0 = pstile()
        pmtA = pmtA0[0:128, 0:C]
        nc.tensor.matmul(pmtA, a0[:, 0:128], wp0[:, :], start=True, stop=False)
        nc.tensor.matmul(pmtA, a1[:, 0:128], wp1[:, :], start=False, stop=True)
        nc.scalar.copy(mt8[0:128, 0:C], pmtA)
        nc.vector.scalar_tensor_tensor(mtr8[0:128, 0:C], mt8[0:128, 0:C], -1.0,
                                       pmtA, A.mult, A.add)
        pmtB0 = pstile()
        pmtB = pmtB0[0:64, 0:C]
        nc.tensor.matmul(pmtB, a0[:, 128:192], wp0[:, :], start=True, stop=False)
        nc.tensor.matmul(pmtB, a1[:, 128:192], wp1[:, :], start=False, stop=True)
        nc.scalar.copy(mt8[0:64, C:2 * C], pmtB)
        nc.vector.scalar_tensor_tensor(mtr8[0:64, C:2 * C], mt8[0:64, C:2 * C],
                                       -1.0, pmtB, A.mult, A.add)
        mt8v = mt8[:, :].rearrange("p (j m) -> p j m", j=2)
        mtr8v = mtr8[:, :].rearrange("p (j m) -> p j m", j=2)

        # ---------- out = M @ v (fp8 DR: M8 v8 + Mr v8 + M8 vr, scaled 1/8) ----
        CHUNK = 1024
        vp = vout8[:, :].ap[0][0]
        vrp = voutr8[:, :].ap[0][0]
        _oe = [0]

        def cpy_scaled(dst, src):
            _oe[0] += 1
            if _oe[0] % 2 == 0:
                nc.scalar.activation(dst, src, AF.Copy, bias=0.0, scale=0.125)
            else:
                nc.vector.tensor_scalar_mul(dst, src, 0.125)

        oa = ob = None
        for t0 in range(0, HW, 512):
          with nc.named_scope("mv"):
            if t0 % CHUNK == 0:
                oa = opool.tile([128, CHUNK], f16, tag="oa")
                ob = opool.tile([64, CHUNK], f16, tag="ob")
            c0 = t0 % CHUNK
            rhs8 = ap_dims(vout8[:, :], [[vp, 128], [HW, 2], [1, 512]],
                           extra_offset=t0)
            rhsr = ap_dims(voutr8[:, :], [[vrp, 128], [HW, 2], [1, 512]],
                           extra_offset=t0)
            pa = pstile()
            nc.tensor.matmul(pa[:, :], mt8v[:, :, 0:128], rhs8,
                             start=True, stop=False, perf_mode=DR)
            nc.tensor.matmul(pa[:, :], mtr8v[:, :, 0:128], rhs8,
                             start=False, stop=False, perf_mode=DR)
            nc.tensor.matmul(pa[:, :], mt8v[:, :, 0:128], rhsr,
                             start=False, stop=True, perf_mode=DR)
            cpy_scaled(oa[:, c0:c0 + 512], pa[:, :])
            pb0 = pstile()
            pb = pb0[0:64, 0:512]
            nc.tensor.matmul(pb, mt8v[:, :, 128:192], rhs8,
                             start=True, stop=False, perf_mode=DR)
            nc.tensor.matmul(pb, mtr8v[:, :, 128:192], rhs8,
                             start=False, stop=False, perf_mode=DR)
            nc.tensor.matmul(pb, mt8v[:, :, 128:192], rhsr,
                             start=False, stop=True, perf_mode=DR)
            cpy_scaled(ob[:, c0:c0 + 512], pb)
            if t0 % CHUNK == CHUNK - 512:
                b0 = t0 + 512 - CHUNK
                nc.sync.dma_start(outd[0:128, b0:b0 + CHUNK], oa[:, :])
                nc.gpsimd.dma_start(outd[128:192, b0:b0 + CHUNK], ob[:, :])

    nc.compile()
    return nc


def _host_consts(w_qkv, w_dw, w_proj, temperature):
    import ml_dtypes
    f8 = ml_dtypes.float8_e4m3

    wq = np.asarray(w_qkv, np.float32)            # [576, 192]
    wd = np.asarray(w_dw, np.float32).reshape(3 * C, 3, 3)
    wpT = np.ascontiguousarray(np.asarray(w_proj, np.float32).T)

    # tap index -> (dy, dx)
    # wq8 [128, 4, 2, 128]
    wq8 = np.zeros((128, 4, 2, 128), np.float32)
    dwq8 = np.zeros((128, 4, 5, 2, 128), np.float32)
    for b in range(4):
        for sb, chb in _slot_groups(b):
            for i in range(CD):
                ch = chb + i
                s = sb + i
                wq8[0:128, b, 0, s] = wq[ch, 0:128]
                wq8[64:128, b, 1, s] = wq[ch, 128:192]
                for t, (dy, dx) in enumerate(TAPS):
                    dwq8[s, b, t // 2, t % 2, s] = wd[ch, dy + 1, dx + 1]

    # v 1x1 weights: out-blocks [128]+[64] packed [128, 2*128 + 2*64]
    wv = np.zeros((128, 2 * 128 + 2 * 64), np.float32)
    for c in range(128):
        ch = 384 + c
        wv[0:128, c] = wq[ch, 0:128]
        wv[64:128, 128 + c] = wq[ch, 128:192]
    for c in range(64):
        ch = 384 + 128 + c
        wv[0:128, 256 + c] = wq[ch, 0:128]
        wv[64:128, 320 + c] = wq[ch, 128:192]
    wv8q = wv.astype(f8)
    wvr8 = (wv - wv8q.astype(np.float32)).astype(f8)

    # v dw weights in fp8 main + residual
    wdv = wd[384:576]                                 # [192, 3, 3]
    wdv_taps = np.stack([wdv[:, dy + 1, dx + 1] for (dy, dx) in TAPS],
                        axis=1)                       # [192, 9]
    w8v = wdv_taps.astype(f8).astype(np.float32)
    wrv = (wdv_taps - w8v).astype(f8).astype(np.float32)
    dwvA8 = np.zeros((128, 9, 2, 128), np.float32)
    dwvrA8 = np.zeros((128, 5, 2, 128), np.float32)
    dwvB8 = np.zeros((128, 9, 2, 64), np.float32)
    for s in range(128):
        for t in range(9):
            dwvA8[s, t, 0, s] = w8v[s, t]
            dwvA8[s, t, 1, s] = wrv[s, t]
        for p in range(5):
            for j in range(2):
                if 2 * p + j < 9:
                    dwvrA8[s, p, j, s] = w8v[s, 2 * p + j]
    for p in range(64):
        ch = 128 + p
        for t in range(9):
            dwvB8[p, t, 0, p] = w8v[ch, t]
            dwvB8[p, t, 1, p] = wrv[ch, t]
            dwvB8[64 + p, t, 0, p] = w8v[ch, t]

    gmask = np.zeros((128, 4 * 96), np.float32)
    for sb, cc in ((0, 0), (32, 24), (64, 48), (96, 72)):
        for i in range(CD):
            for b in range(4):
                gmask[sb + i, 96 * b + cc + i] = 1.0

    tmap = np.ones((128, 4), np.float32)
    tf = np.asarray(temperature, np.float32).reshape(-1)
    for b in range(4):
        tmap[32:56, b] = tf[2 * b]
        tmap[96:120, b] = tf[2 * b + 1]

    return dict(
        wq8=wq8.reshape(128, -1).astype(f8),
        dwq8=dwq8.reshape(128, -1).astype(f8),
        wv8=wv8q,
        wvr8=wvr8,
        dwvA8=dwvA8.reshape(128, -1).astype(f8),
        dwvrA8=dwvrA8.reshape(128, -1).astype(f8),
        dwvB8=dwvB8.reshape(128, -1).astype(f8),
        wpT=wpT * 8.0,
        gmask=gmask,
        tmap=tmap,
        eye=_eye_slim(),
    )


def _eye_slim():
    e = np.zeros((128, 24), np.float32)
    for kb in (32, 96):
        for i in range(24):
            e[kb + i, i] = 1.0
    return e


def kernel(x, w_qkv, w_dw, w_proj, temperature, _trace=False):
    import ml_dtypes
    from concourse.bass_utils import run_bass_kernel_spmd

    if "nc" not in _CACHE:
        _CACHE["nc"] = _build()
    nc = _CACHE["nc"]

    consts = _host_consts(w_qkv, w_dw, w_proj, temperature)
    xr = np.ascontiguousarray(np.asarray(x, np.float32).reshape(NCORES, C, HW))
    x8 = xr.astype(ml_dtypes.float8_e4m3)
    xr8 = (xr - x8.astype(np.float32)).astype(ml_dtypes.float8_e4m3)
    in_maps = []
    for bb in range(NCORES):
        m = {"x8": x8[bb], "xr8": xr8[bb],
             "zpad": np.zeros((64, HW), ml_dtypes.float8_e4m3)}
        m.update(consts)
        in_maps.append(m)

    try:
        br = run_bass_kernel_spmd(nc, in_maps, core_ids=list(range(NCORES)),
                                  trace=_trace)
    except ModuleNotFoundError:
        br = run_bass_kernel_spmd(nc, in_maps, core_ids=list(range(NCORES)),
                                  trace=False)
    out = np.stack([np.asarray(r["out"], dtype=np.float32) for r in br.results],
                   axis=0).reshape(NCORES, C, H, W)
    _CACHE["last_results"] = br
    return out



# revision 22
# speedup vs baseline: 1.0519x; 1.0004x over previous
"""Trainium2 Bass kernel for Restormer-style transposed (channel) attention, v2.

Per-core (1 of 8 batch elements), built around the TimelineSim cost model
(matmul cost = output free-size; fp8e4m3 DoubleRow = 0.5 cyc/col):

  q/k path (errors wash out through the softmax normalization):
    z_qk = Wqk8 @ x8            fp8 DoubleRow, 192-contraction in 1 instr
    dwconv 3x3                  5 DR diag tap-pair matmuls per block (2.5 cyc/px)
    -> bf16 qb -> xbar DMA transpose -> [px, slot] qki tiles
    gram G += qki^T qki         bf16, compact-col strided rhs
  v path (kept accurate):
    z_v = Wv @ x16              bf16
    dwconv = DR(fp8(z_v)) + DR(fp8(z_v - fp8(z_v)))   exact to ~0.2%
    vout fp16 resident in SBUF
  tail: norms from gram diag, softmax per head, M^T = A_bd^T Wproj^T,
        out = M @ vout streamed to HBM.

Slot layout (32-aligned, 4 blocks of 128):
  block b: [q_{2b} 0:24 | pad | k_{2b} 32:56 | pad | q_{2b+1} 64:88 | pad |
            k_{2b+1} 96:120 | pad]
"""
import numpy as np

NUM_HEADS = 8
C = 192
H = W = 128
HW = H * W
CD = 24
NCORES = 8
SLAB = 16
NSLABS = H // SLAB
EPS = 1e-12
PW = W + 4
IMG0 = 2
NR = SLAB + 2          # z8 slab rows incl halo

# tap order chosen so DR pairs have EVEN offset deltas (hw requirement):
# pairs: ((-1,-1),(-1,1)) ((0,-1),(0,1)) ((1,-1),(1,1)) ((-1,0),(0,0)) ((1,0),zero)
TAPS = [(-1, -1), (-1, 1), (0, -1), (0, 1), (1, -1), (1, 1), (-1, 0), (0, 0), (1, 0)]

# per-block slot groups: (slot_base, qkv_ch_base)
def _slot_groups(b):
    return [(0, 48 * b), (32, 192 + 48 * b), (64, 48 * b + 24), (96, 192 + 48 * b + 24)]


_CACHE = {}


def _stripes(ncols):
    """split ncols into row-aligned stripes of >=256 (multiples of 128)."""
    out = []
    rem = ncols
    while rem > 0:
        t = min(512, rem)
        if rem - t == 128:
            t = 384
        out.append(t)
        rem -= t
    return out


def _build():
    import concourse.bass as bass
    import concourse.mybir as mybir
    import concourse.tile as tile
    from concourse import bacc
    from contextlib import ExitStack
    import bass_rust

    dt = mybir.dt
    A = mybir.AluOpType
    AF = mybir.ActivationFunctionType
    AX = mybir.AxisListType
    DR = mybir.MatmulPerfMode.DoubleRow
    f32, bf16, f16, f8, f32r = dt.float32, dt.bfloat16, dt.float16, dt.float8e4, dt.float32r

    def ap_dims(ap, dims, extra_offset=0):
        c = ap.copy()
        c.ap = bass_rust.VecI64Pair(dims)
        c.offset = ap.offset + extra_offset
        return c

    nc = bacc.Bacc("TRN2", num_devices=NCORES)

    x8d = nc.dram_tensor("x8", [C, HW], f8, kind="ExternalInput").ap()
    xr8d = nc.dram_tensor("xr8", [C, HW], f8, kind="ExternalInput").ap()
    wq8d = nc.dram_tensor("wq8", [128, 4 * 2 * 128], f8, kind="ExternalInput").ap()
    dwq8d = nc.dram_tensor("dwq8", [128, 4 * 5 * 2 * 128], f8, kind="ExternalInput").ap()
    # v 1x1 weights, out-blocks [128]+[64]: [.., 0:256]=lo j2 m128, [.., 256:384]=hi j2 m64
    wv8d = nc.dram_tensor("wv8", [128, 2 * 128 + 2 * 64], f8, kind="ExternalInput").ap()
    wvr8d = nc.dram_tensor("wvr8", [128, 2 * 128 + 2 * 64], f8, kind="ExternalInput").ap()
    # v dw: A = dual-weight (w8,wr) per tap for ch0-127; rA = tap-paired w8 for r ch0-127;
    # B = mixed block (rows 0-63: z8v ch128-191 dual; rows 64-127: r ch128-191 single w8)
    dwvA8d = nc.dram_tensor("dwvA8", [128, 9 * 2 * 128], f8, kind="ExternalInput").ap()
    dwvrA8d = nc.dram_tensor("dwvrA8", [128, 5 * 2 * 128], f8, kind="ExternalInput").ap()
    dwvB8d = nc.dram_tensor("dwvB8", [128, 9 * 2 * 64], f8, kind="ExternalInput").ap()
    zpadd = nc.dram_tensor("zpad", [64, HW], f8, kind="ExternalInput").ap()
    wpTd = nc.dram_tensor("wpT", [C, C], f32, kind="ExternalInput").ap()
    mskd = nc.dram_tensor("gmask", [128, 4 * 96], f32, kind="ExternalInput").ap()
    tmpd = nc.dram_tensor("tmap", [128, 4], f32, kind="ExternalInput").ap()
    eyed = nc.dram_tensor("eye", [128, 24], f32, kind="ExternalInput").ap()
    outd = nc.dram_tensor("out", [C, HW], f16, kind="ExternalOutput").ap()

    with tile.TileContext(nc) as tc:
      with ExitStack() as _es:
        cpool = _es.enter_context(tc.tile_pool(name="const", bufs=1))
        xpool = _es.enter_context(tc.tile_pool(name="xin", bufs=2))
        zpool = _es.enter_context(tc.tile_pool(name="zst", bufs=2))
        qpool = _es.enter_context(tc.tile_pool(name="qbt", bufs=2))
        kpool = _es.enter_context(tc.tile_pool(name="qki", bufs=2))
        vpool = _es.enter_context(tc.tile_pool(name="vout", bufs=1))
        mpool = _es.enter_context(tc.tile_pool(name="sm", bufs=2))
        apool = _es.enter_context(tc.tile_pool(name="abd", bufs=1))
        opool = _es.enter_context(tc.tile_pool(name="outs", bufs=3))
        psP = _es.enter_context(tc.tile_pool(name="psP", bufs=7, space="PSUM"))
        psG = _es.enter_context(tc.tile_pool(name="psG", bufs=1, space="PSUM"))

        def pstile():
            return psP.tile([128, 512], f32, tag="ps", name="ps")

        # ---------- constants (ACT dma queue; x streams on sync) ----------
        wq8 = cpool.tile([128, 4 * 2 * 128], f8, tag="wq8")
        nc.scalar.dma_start(wq8[:, :], wq8d[:, :])
        wv8 = cpool.tile([128, 2 * 128 + 2 * 64], f8, tag="wv8")
        nc.scalar.dma_start(wv8[:, :], wv8d[:, :])
        wvr8 = cpool.tile([128, 2 * 128 + 2 * 64], f8, tag="wvr8")
        nc.scalar.dma_start(wvr8[:, :], wvr8d[:, :])
        dwq8 = cpool.tile([128, 4 * 5 * 2 * 128], f8, tag="dwq8")
        nc.scalar.dma_start(dwq8[:, :], dwq8d[:, :])
        dwvA8 = cpool.tile([128, 9 * 2 * 128], f8, tag="dwvA8")
        nc.scalar.dma_start(dwvA8[:, :], dwvA8d[:, :])
        dwvrA8 = cpool.tile([128, 5 * 2 * 128], f8, tag="dwvrA8")
        nc.scalar.dma_start(dwvrA8[:, :], dwvrA8d[:, :])
        dwvB8 = cpool.tile([128, 9 * 2 * 64], f8, tag="dwvB8")
        nc.scalar.dma_start(dwvB8[:, :], dwvB8d[:, :])
        msk = cpool.tile([128, 4 * 96], f32, tag="msk")
        nc.scalar.dma_start(msk[:, :], mskd[:, :])
        tmap = cpool.tile([128, 4], f32, tag="tmap")
        nc.scalar.dma_start(tmap[:, :], tmpd[:, :])
        eye = cpool.tile([128, 24], f32, tag="eye")
        nc.scalar.dma_start(eye[:, :], eyed[:, :])
        wp0 = cpool.tile([96, C], f32, tag="wp0")
        nc.scalar.dma_start(wp0[:, :], wpTd[0:96, :])
        wp1 = cpool.tile([96, C], f32, tag="wp1")
        nc.scalar.dma_start(wp1[:, :], wpTd[96:192, :])
        # warm the ACT function tables (Sqrt/Exp) off the critical path
        warm = cpool.tile([1, 2], f32, tag="warm")
        nc.scalar.sqrt(warm[0:1, 0:1], tmap[0:1, 0:1])
        nc.scalar.activation(warm[0:1, 1:2], tmap[0:1, 0:1], AF.Exp,
                             bias=0.0, scale=1.0)

        wq8v = wq8[:, :].rearrange("p (b j s) -> p b j s", b=4, j=2)
        dwq8v = dwq8[:, :].rearrange("p (b k j s) -> p b k j s", b=4, k=5, j=2)
        wv8lo = wv8[:, 0:256].rearrange("p (j s) -> p j s", j=2)
        wv8hi = wv8[:, 256:384].rearrange("p (j s) -> p j s", j=2)
        wvr8lo = wvr8[:, 0:256].rearrange("p (j s) -> p j s", j=2)
        wvr8hi = wvr8[:, 256:384].rearrange("p (j s) -> p j s", j=2)
        dwvA8v = dwvA8[:, :].rearrange("p (t j s) -> p t j s", t=9, j=2)
        dwvrA8v = dwvrA8[:, :].rearrange("p (t j s) -> p t j s", t=5, j=2)
        dwvB8v = dwvB8[:, :].rearrange("p (t j s) -> p t j s", t=9, j=2)

        # vout in fp8 main+residual; contraction j-split 128+64 for M@v DR
        # (j1 rows 64-127 are zero pad, memset once on Pool)
        vout8 = vpool.tile([128, 2 * HW], f8, tag="vout8", name="vout8")
        voutr8 = vpool.tile([128, 2 * HW], f8, tag="voutr8", name="voutr8")
        nc.scalar.dma_start(vout8[64:128, HW:2 * HW], zpadd[:, :])
        nc.scalar.dma_start(voutr8[64:128, HW:2 * HW], zpadd[:, :])
        gram = psG.tile([128, 4 * 96], f32, tag="g")

        # evac engine round-robin (ACT / DVE alternating)
        _ev = [0]
        def cpy(dst, src):
            _ev[0] += 1
            if _ev[0] % 2 == 0:
                nc.scalar.copy(dst, src)
            else:
                nc.vector.tensor_copy(dst, src)

        # slab state carried across pipeline iterations
        z8s, zv8s, qbs, qkis, xts = {}, {}, {}, {}, {}

        def _slabmeta(s):
            r0 = SLAB * s
            lo, hi = max(0, r0 - 1), min(H - 1, r0 + SLAB)
            nrows = hi - lo + 1
            return r0, lo, nrows, lo - (r0 - 1)

        def emit_loads(s, half=None):
            r0, lo, nrows, slot0 = _slabmeta(s)
            ncols = nrows * W
            col0 = lo * W
            if half in (None, 0):
                xs8 = xpool.tile([128, 2 * ncols], f8, tag="xs8")
                nc.sync.dma_start(xs8[:, 0:ncols], x8d[0:128, col0:col0 + ncols])
                # k-tile 1 holds x channels 64..191 (rows 0-63 have zero weights)
                nc.sync.dma_start(xs8[:, ncols:2 * ncols],
                                  x8d[64:192, col0:col0 + ncols])
                xts[s] = [xs8, None, ncols, slot0]
            if half in (None, 1):
                xsr8 = xpool.tile([128, 2 * ncols], f8, tag="xsr8")
                nc.sync.dma_start(xsr8[:, 0:ncols], xr8d[0:128, col0:col0 + ncols])
                nc.sync.dma_start(xsr8[:, ncols:2 * ncols],
                                  xr8d[64:192, col0:col0 + ncols])
                xts[s][1] = xsr8

        def emit_qkv_chunk(s, c):
            r0, lo, nrows, slot0 = _slabmeta(s)
            ncols = nrows * W
            if c == 0:
                z8 = [zpool.tile([128, NR * PW], f8, tag=f"z8_{b}", name=f"z8_{b}")
                      for b in range(4)]
                # zA: z8v ch0-127; rA: r ch0-127; B: rows 0-63 z8v ch128-191,
                # rows 64-127 r ch128-191
                zv8 = [zpool.tile([128, NR * PW], f8, tag=f"zv8_{v}", name=f"zv8_{v}")
                       for v in range(3)]
                if s < 2:
                    for t in z8 + zv8:
                        tv = t[:, :].rearrange("p (r w) -> p r w", w=PW)
                        nc.gpsimd.memset(tv[:, :, 0:IMG0], 0.0)
                        nc.gpsimd.memset(tv[:, :, IMG0 + W:PW], 0.0)
                if s == 0:
                    for t in z8 + zv8:
                        nc.gpsimd.memset(t[:, 0:PW], 0.0)
                if s == NSLABS - 1:
                    for t in z8 + zv8:
                        nc.gpsimd.memset(t[:, (NR - 1) * PW:NR * PW], 0.0)
                z8s[s], zv8s[s] = z8, zv8
            xs8, xsr8, ncols, slot0 = xts[s]
            z8, zv8 = z8s[s], zv8s[s]
            zA, rA, zB = zv8
            xs8p = xs8[:, :].ap[0][0]
            xsr8p = xsr8[:, :].ap[0][0]
            strs = _stripes(ncols)
            todo = [c] if c < 3 else [3] + list(range(4, len(strs)))
            for si in todo:
                tw = strs[si]
                t0 = sum(strs[:si])
                row0 = t0 // W
                nr = tw // W
                for b in range(4):
                  with nc.named_scope(f"qkvqk{s}"):
                    ps = pstile()
                    rhs = ap_dims(xs8[:, :], [[xs8p, 128], [ncols, 2], [1, tw]],
                                  extra_offset=t0)
                    nc.tensor.matmul(ps[0:128, 0:tw], wq8v[:, b, :, :], rhs,
                                     start=True, stop=True, perf_mode=DR)
                    zview = z8[b][:, :].rearrange("p (r w) -> p r w", w=PW)
                    cpy(zview[:, slot0 + row0: slot0 + row0 + nr, IMG0:IMG0 + W],
                        ps[0:128, 0:tw].rearrange("p (r w) -> p r w", w=W))
                with nc.named_scope(f"qkvv{s}"):
                    rhs8 = ap_dims(xs8[:, :], [[xs8p, 128], [ncols, 2], [1, tw]],
                                   extra_offset=t0)
                    rhsr = ap_dims(xsr8[:, :], [[xsr8p, 128], [ncols, 2], [1, tw]],
                                   extra_offset=t0)
                    psl = pstile()
                    nc.tensor.matmul(psl[:, 0:tw], wv8lo, rhs8,
                                     start=True, stop=False, perf_mode=DR)
                    nc.tensor.matmul(psl[:, 0:tw], wv8lo, rhsr,
                                     start=False, stop=False, perf_mode=DR)
                    nc.tensor.matmul(psl[:, 0:tw], wvr8lo, rhs8,
                                     start=False, stop=True, perf_mode=DR)
                    psh0 = pstile()
                    psh = psh0[0:64, 0:tw]
                    nc.tensor.matmul(psh, wv8hi, rhs8,
                                     start=True, stop=False, perf_mode=DR)
                    nc.tensor.matmul(psh, wv8hi, rhsr,
                                     start=False, stop=False, perf_mode=DR)
                    nc.tensor.matmul(psh, wvr8hi, rhs8,
                                     start=False, stop=True, perf_mode=DR)
                    zAv = zA[:, :].rearrange("p (r w) -> p r w", w=PW)
                    rAv = rA[:, :].rearrange("p (r w) -> p r w", w=PW)
                    zBv = zB[:, :].rearrange("p (r w) -> p r w", w=PW)
                    zw = zAv[:, slot0 + row0: slot0 + row0 + nr, IMG0:IMG0 + W]
                    rw = rAv[:, slot0 + row0: slot0 + row0 + nr, IMG0:IMG0 + W]
                    bzw = zBv[0:64, slot0 + row0: slot0 + row0 + nr, IMG0:IMG0 + W]
                    brw = zBv[64:128, slot0 + row0: slot0 + row0 + nr, IMG0:IMG0 + W]
                    pslw = psl[:, 0:tw].rearrange("p (r w) -> p r w", w=W)
                    pshw = psh.rearrange("p (r w) -> p r w", w=W)
                    nc.scalar.copy(zw, pslw)
                    nc.vector.scalar_tensor_tensor(rw, zw, -1.0, pslw, A.mult, A.add)
                    nc.scalar.copy(bzw, pshw)
                    nc.vector.scalar_tensor_tensor(brw, bzw, -1.0, pshw,
                                                   A.mult, A.add)
            if c == 3:
                xts.pop(s)

        def emit_dwqk_chunk(s, c):
            g = c
            if c == 0:
                qbs[s] = [qpool.tile([128, 2 * 4 * W], bf16, tag=f"qb{b}",
                                     name=f"qb{b}") for b in range(4)]
                qkis[s] = [kpool.tile([128, SLAB * W], bf16, tag=f"qki{b}",
                                      name=f"qki{b}") for b in range(4)]
            z8, qb, qki = z8s[s], qbs[s], qkis[s]
            for b in range(4):
              with nc.named_scope(f"dwqk{s}"):
                zp = z8[b][:, :].ap[0][0]
                pd = pstile()
                for p in range(5):
                    dy0, dx0 = TAPS[2 * p]
                    o0 = (4 * g + 1 + dy0) * PW + IMG0 + dx0
                    if 2 * p + 1 < 9:
                        dy1, dx1 = TAPS[2 * p + 1]
                        o1 = (4 * g + 1 + dy1) * PW + IMG0 + dx1
                    else:
                        o1 = o0 + 2
                    rhs = ap_dims(z8[b][:, :],
                                  [[zp, 128], [o1 - o0, 2], [PW, 4], [1, W]],
                                  extra_offset=o0)
                    nc.tensor.matmul(pd[:, :], dwq8v[:, b, p, :, :], rhs,
                                     start=(p == 0), stop=(p == 4),
                                     perf_mode=DR)
                cpy(qb[b][:, (g % 2) * 512:(g % 2) * 512 + 512], pd[:, :])
                if g % 2 == 1:
                    h = g // 2
                    qkv_view = qki[b][:, h * 8 * W:(h + 1) * 8 * W].rearrange(
                        "p (t s) -> p t s", t=8)
                    nc.sync.dma_start_transpose(qkv_view, qb[b][:, :])
            if c == 3:
                z8s.pop(s)

        def emit_dwv_chunk(s, c):
            r0 = SLAB * s
            g = c
            zA, rA, zB = zv8s[s]
            with nc.named_scope(f"dwv{s}"):
                zp = zA[:, :].ap[0][0]
                rp = rA[:, :].ap[0][0]
                bp = zB[:, :].ap[0][0]
                psV = pstile()
                psV20 = pstile()
                psV2 = psV20[0:64, 0:512]
                # A: 9 dual-weight taps (w8 in j0, wr in j1, j-stride 0)
                for t in range(9):
                    dy, dx = TAPS[t]
                    ot = (4 * g + 1 + dy) * PW + IMG0 + dx
                    rhs = ap_dims(zA[:, :],
                                  [[zp, 128], [0, 2], [PW, 4], [1, W]],
                                  extra_offset=ot)
                    nc.tensor.matmul(psV[:, :], dwvA8v[:, t, :, :], rhs,
                                     start=(t == 0), stop=False, perf_mode=DR)
                # rA: 5 tap-paired w8 instrs, accumulate onto psV
                for p in range(5):
                    dy0, dx0 = TAPS[2 * p]
                    o0 = (4 * g + 1 + dy0) * PW + IMG0 + dx0
                    if 2 * p + 1 < 9:
                        dy1, dx1 = TAPS[2 * p + 1]
                        o1 = (4 * g + 1 + dy1) * PW + IMG0 + dx1
                    else:
                        o1 = o0 + 2
                    rhs = ap_dims(rA[:, :],
                                  [[rp, 128], [o1 - o0, 2], [PW, 4], [1, W]],
                                  extra_offset=o0)
                    nc.tensor.matmul(psV[:, :], dwvrA8v[:, p, :, :], rhs,
                                     start=False, stop=(p == 4), perf_mode=DR)
                # B: 9 taps; rows 0-63 dual-weight z8v hi, rows 64-127 r hi (w8)
                for t in range(9):
                    dy, dx = TAPS[t]
                    ot = (4 * g + 1 + dy) * PW + IMG0 + dx
                    rhs = ap_dims(zB[:, :],
                                  [[bp, 128], [0, 2], [PW, 4], [1, W]],
                                  extra_offset=ot)
                    nc.tensor.matmul(psV2, dwvB8v[:, t, :, :], rhs,
                                     start=(t == 0), stop=(t == 8), perf_mode=DR)
                # evac into vout8/voutr8 (j-split 128+64): ch0-127 | ch128-191
                c0 = (r0 + 4 * g) * W
                nc.scalar.copy(vout8[0:128, c0:c0 + 512], psV[:, :])
                nc.vector.scalar_tensor_tensor(
                    voutr8[0:128, c0:c0 + 512], vout8[0:128, c0:c0 + 512],
                    -1.0, psV[:, :], A.mult, A.add)
                nc.scalar.copy(vout8[0:64, HW + c0:HW + c0 + 512], psV2)
                nc.vector.scalar_tensor_tensor(
                    voutr8[0:64, HW + c0:HW + c0 + 512],
                    vout8[0:64, HW + c0:HW + c0 + 512],
                    -1.0, psV2, A.mult, A.add)
            if c == 3:
                zv8s.pop(s)

        def emit_gram_chunk(s, c):
            qki = qkis[s]
            for u in range(4 * c, 4 * c + 4):
                g_idx = SLAB * s + u
                for b in range(4):
                  with nc.named_scope(f"gram{s}"):
                    lhsT = qki[b][:, u * W:(u + 1) * W]
                    rhs = ap_dims(qki[b][:, :],
                                  [[qki[b][:, :].ap[0][0], 128], [32, 4], [1, 24]],
                                  extra_offset=u * W)
                    nc.tensor.matmul(gram[:, 96 * b:96 * (b + 1)], lhsT, rhs,
                                     start=(g_idx == 0), stop=(g_idx == H - 1),
                                     skip_group_check=True)
            if c == 3:
                qbs.pop(s), qkis.pop(s)

        # ---------- pipeline (flat chunk stream with per-stage lags) ----------
        NCHUNK = 4 * NSLABS
        LAGD, LAGG = 2, 6
        emit_loads(0)
        for pos in range(NCHUNK + LAGG + 1):
            s, c = divmod(pos, 4)
            if c in (1, 2) and s + 1 < NSLABS:
                emit_loads(s + 1, half=c - 1)
            if pos < NCHUNK:
                emit_qkv_chunk(s, c)
            p = pos - LAGD
            if 0 <= p < NCHUNK:
                s2, c2 = divmod(p, 4)
                emit_dwqk_chunk(s2, c2)
                if s2 < NSLABS - 1:
                    emit_dwv_chunk(s2, c2)
            p = pos - LAGG
            if 0 <= p < NCHUNK:
                s3, c3 = divmod(p, 4)
                emit_gram_chunk(s3, c3)
                if s3 == NSLABS - 1 and c3 in (2, 3):
                    emit_dwv_chunk(NSLABS - 1, c3 - 2)

        # ---------- norms ----------
        gm = mpool.tile([128, 4 * 96], f32, tag="gm", bufs=1)
        nc.vector.tensor_tensor(gm[:, :], gram[:, :], msk[:, :], A.mult)
        s_sb = mpool.tile([128, 4], f32, tag="ssb")
        nc.vector.tensor_reduce(s_sb[:, :],
                                gm[:, :].rearrange("p (g c) -> p g c", g=4),
                                AX.X, A.add)
        ns = mpool.tile([128, 4], f32, tag="ns")
        nc.scalar.sqrt(ns[:, :], s_sb[:, :])
        nsc = mpool.tile([128, 4], f32, tag="nsc")
        nc.vector.tensor_scalar_max(nsc[:, :], ns[:, :], EPS)
        ry = mpool.tile([128, 4], f32, tag="ry")
        nc.vector.reciprocal(ry[:, :], nsc[:, :])
        t1 = mpool.tile([128, 4], f32, tag="t1")
        nc.vector.tensor_tensor(t1[:, :], s_sb[:, :], ry[:, :], A.mult)
        t2 = mpool.tile([128, 4], f32, tag="t2")
        nc.vector.tensor_add(t2[:, :], nsc[:, :], t1[:, :])
        ns2 = mpool.tile([128, 4], f32, tag="ns2")
        nc.vector.tensor_scalar_mul(ns2[:, :], t2[:, :], 0.5)
        ns3 = mpool.tile([128, 4], f32, tag="ns3")
        nc.vector.tensor_scalar_max(ns3[:, :], ns2[:, :], EPS)
        rn = mpool.tile([128, 4], f32, tag="rn")
        nc.vector.reciprocal(rn[:, :], ns3[:, :])
        rkt = mpool.tile([128, 4], f32, tag="rkt")
        nc.vector.tensor_tensor(rkt[:, :], rn[:, :], tmap[:, :], A.mult)
        rq = mpool.tile([24, 8], f32, tag="rq")
        nc.sync.dma_start(rq[0:24, 1:8:2], rn[64:88, 0:4])

        # ---------- softmax + A blockdiag ----------
        a0 = apool.tile([96, C], f32, tag="a0")
        a1 = apool.tile([96, C], f32, tag="a1")
        nc.vector.memset(a0[:, :], 0.0)
        nc.vector.memset(a1[:, :], 0.0)
        bt = mpool.tile([128, 8 * CD], f32, tag="bt", bufs=1)
        AF_ = AF
        for h in range(NUM_HEADS):
            b = h // 2
            kbase = 32 if h % 2 == 0 else 96
            qcol = 0 if h % 2 == 0 else 48
            nc.vector.tensor_scalar_mul(
                bt[kbase:kbase + CD, CD * h:CD * (h + 1)],
                gram[kbase:kbase + CD, 96 * b + qcol:96 * b + qcol + CD],
                rkt[kbase:kbase + CD, b:b + 1])
            ptr0 = pstile()
            ptr = ptr0[0:CD, 0:CD]
            nc.tensor.transpose(ptr,
                                bt[kbase:kbase + CD, CD * h:CD * (h + 1)],
                                eye[kbase:kbase + CD, 0:CD],
                                tile_position=(kbase, 0))
            es = mpool.tile([CD, CD], f32, tag="es")
            se = mpool.tile([CD, 1], f32, tag="se")
            rqh = (rn[0:24, b:b + 1] if h % 2 == 0
                   else rq[0:24, h:h + 1])
            nc.scalar.activation(es[:, :], ptr, AF_.Exp,
                                 bias=0.0, scale=rqh,
                                 accum_out=se[:, :])
            rse = mpool.tile([CD, 1], f32, tag="rse")
            nc.vector.reciprocal(rse[:, :], se[:, :])
            ah = mpool.tile([CD, CD], f32, tag="ah")
            nc.vector.tensor_scalar_mul(ah[:, :], es[:, :], rse[0:CD, 0:1])
            adst = a0 if h < 4 else a1
            r0 = 24 * (h % 4)
            nc.sync.dma_start(adst[r0:r0 + CD, CD * h:CD * (h + 1)], ah[:, :])

        for c in (2, 3):
            emit_dwv_chunk(NSLABS - 1, c)

        # ---------- M^T = A_bd^T @ (8*W_proj^T) (fp32), then fp8 + residual ----
        # wpT is pre-scaled x8 host-side so M8/Mr stay clear of the fp8
        # subnormal floor; the x8 is undone at the out-evac (scale 1/8).
        # mt8 rows = v-ch: j0 rows 0-127 = ch0-127, j1 rows 0-63 = ch128-191,
        # j1 rows 64-127 zero (matches vout8 pad).
        mt8 = cpool.tile([128, 2 * C], f8, tag="mt8")
        mtr8 = cpool.tile([128, 2 * C], f8, tag="mtr8")
        nc.gpsimd.memset(mt8[64:128, C:2 * C], 0.0)
        nc.gpsimd.memset(mtr8[64:128, C:2 * C], 0.0)
        pmtA0 = pstile()
        pmtA = pmtA0[0:128, 0:C]
        nc.tensor.matmul(pmtA, a0[:, 0:128], wp0[:, :], start=True, stop=False)
        nc.tensor.matmul(pmtA, a1[:, 0:128], wp1[:, :], start=False, stop=True)
        nc.scalar.copy(mt8[0:128, 0:C], pmtA)
        nc.vector.scalar_tensor_tensor(mtr8[0:128, 0:C], mt8[0:128, 0:C], -1.0,
                                       pmtA, A.mult, A.add)
        pmtB0 = pstile()
        pmtB = pmtB0[0:64, 0:C]
        nc.tensor.matmul(pmtB, a0[:, 128:192], wp0[:, :], start=True, stop=False)
        nc.tensor.matmul(pmtB, a1[:, 128:192], wp1[:, :], start=False, stop=True)
        nc.scalar.copy(mt8[0:64, C:2 * C], pmtB)
        nc.vector.scalar_tensor_tensor(mtr8[0:64, C:2 * C], mt8[0:64, C:2 * C],
                                       -1.0, pmtB, A.mult, A.add)
        mt8v = mt8[:, :].rearrange("p (j m) -> p j m", j=2)
        mtr8v = mtr8[:, :].rearrange("p (j m) -> p j m", j=2)

        # ---------- out = M @ v (fp8 DR: M8 v8 + Mr v8 + M8 vr, scaled 1/8) ----
        CHUNK = 1024
        vp = vout8[:, :].ap[0][0]
        vrp = voutr8[:, :].ap[0][0]
        _oe = [0]

        def cpy_scaled(dst, src):
            _oe[0] += 1
            if _oe[0] % 2 == 0:
                nc.scalar.activation(dst, src, AF.Copy, bias=0.0, scale=0.125)
            else:
                nc.vector.tensor_scalar_mul(dst, src, 0.125)

        oa = ob = None
        for t0 in range(0, HW, 512):
          with nc.named_scope("mv"):
            if t0 % CHUNK == 0:
                oa = opool.tile([128, CHUNK], f16, tag="oa")
                ob = opool.tile([64, CHUNK], f16, tag="ob")
            c0 = t0 % CHUNK
            rhs8 = ap_dims(vout8[:, :], [[vp, 128], [HW, 2], [1, 512]],
                           extra_offset=t0)
            rhsr = ap_dims(voutr8[:, :], [[vrp, 128], [HW, 2], [1, 512]],
                           extra_offset=t0)
            pa = pstile()
            nc.tensor.matmul(pa[:, :], mt8v[:, :, 0:128], rhs8,
                             start=True, stop=False, perf_mode=DR)
            nc.tensor.matmul(pa[:, :], mtr8v[:, :, 0:128], rhs8,
                             start=False, stop=False, perf_mode=DR)
            nc.tensor.matmul(pa[:, :], mt8v[:, :, 0:128], rhsr,
                             start=False, stop=True, perf_mode=DR)
            cpy_scaled(oa[:, c0:c0 + 512], pa[:, :])
            pb0 = pstile()
            pb = pb0[0:64, 0:512]
            nc.tensor.matmul(pb, mt8v[:, :, 128:192], rhs8,
                             start=True, stop=False, perf_mode=DR)
            nc.tensor.matmul(pb, mtr8v[:, :, 128:192], rhs8,
                             start=False, stop=False, perf_mode=DR)
            nc.tensor.matmul(pb, mt8v[:, :, 128:192], rhsr,
                             start=False, stop=True, perf_mode=DR)
            cpy_scaled(ob[:, c0:c0 + 512], pb)
            if t0 % CHUNK == CHUNK - 512:
                b0 = t0 + 512 - CHUNK
                nc.sync.dma_start(outd[0:128, b0:b0 + CHUNK], oa[:, :])
                nc.gpsimd.dma_start(outd[128:192, b0:b0 + CHUNK], ob[:, :])

    nc.compile()
    return nc


def _host_consts(w_qkv, w_dw, w_proj, temperature):
    import ml_dtypes
    f8 = ml_dtypes.float8_e4m3

    wq = np.asarray(w_qkv, np.float32)            # [576, 192]
    wd = np.asarray(w_dw, np.float32).reshape(3 * C, 3, 3)
    wpT = np.ascontiguousarray(np.asarray(w_proj, np.float32).T)

    # tap index -> (dy, dx)
    # wq8 [128, 4, 2, 128]
    wq8 = np.zeros((128, 4, 2, 128), np.float32)
    dwq8 = np.zeros((128, 4, 5, 2, 128), np.float32)
    for b in range(4):
        for sb, chb in _slot_groups(b):
            for i in range(CD):
                ch = chb + i
                s = sb + i
                wq8[0:128, b, 0, s] = wq[ch, 0:128]
                wq8[64:128, b, 1, s] = wq[ch, 128:192]
                for t, (dy, dx) in enumerate(TAPS):
                    dwq8[s, b, t // 2, t % 2, s] = wd[ch, dy + 1, dx + 1]

    # v 1x1 weights: out-blocks [128]+[64] packed [128, 2*128 + 2*64]
    wv = np.zeros((128, 2 * 128 + 2 * 64), np.float32)
    for c in range(128):
        ch = 384 + c
        wv[0:128, c] = wq[ch, 0:128]
        wv[64:128, 128 + c] = wq[ch, 128:192]
    for c in range(64):
        ch = 384 + 128 + c
        wv[0:128, 256 + c] = wq[ch, 0:128]
        wv[64:128, 320 + c] = wq[ch, 128:192]
    wv8q = wv.astype(f8)
    wvr8 = (wv - wv8q.astype(np.float32)).astype(f8)

    # v dw weights in fp8 main + residual
    wdv = wd[384:576]                                 # [192, 3, 3]
    wdv_taps = np.stack([wdv[:, dy + 1, dx + 1] for (dy, dx) in TAPS],
                        axis=1)                       # [192, 9]
    w8v = wdv_taps.astype(f8).astype(np.float32)
    wrv = (wdv_taps - w8v).astype(f8).astype(np.float32)
    dwvA8 = np.zeros((128, 9, 2, 128), np.float32)
    dwvrA8 = np.zeros((128, 5, 2, 128), np.float32)
    dwvB8 = np.zeros((128, 9, 2, 64), np.float32)
    for s in range(128):
        for t in range(9):
            dwvA8[s, t, 0, s] = w8v[s, t]
            dwvA8[s, t, 1, s] = wrv[s, t]
        for p in range(5):
            for j in range(2):
                if 2 * p + j < 9:
                    dwvrA8[s, p, j, s] = w8v[s, 2 * p + j]
    for p in range(64):
        ch = 128 + p
        for t in range(9):
            dwvB8[p, t, 0, p] = w8v[ch, t]
            dwvB8[p, t, 1, p] = wrv[ch, t]
            dwvB8[64 + p, t, 0, p] = w8v[ch, t]

    gmask = np.zeros((128, 4 * 96), np.float32)
    for sb, cc in ((0, 0), (32, 24), (64, 48), (96, 72)):
        for i in range(CD):
            for b in range(4):
                gmask[sb + i, 96 * b + cc + i] = 1.0

    tmap = np.ones((128, 4), np.float32)
    tf = np.asarray(temperature, np.float32).reshape(-1)
    for b in range(4):
        tmap[32:56, b] = tf[2 * b]
        tmap[96:120, b] = tf[2 * b + 1]

    return dict(
        wq8=wq8.reshape(128, -1).astype(f8),
        dwq8=dwq8.reshape(128, -1).astype(f8),
        wv8=wv8q,
        wvr8=wvr8,
        dwvA8=dwvA8.reshape(128, -1).astype(f8),
        dwvrA8=dwvrA8.reshape(128, -1).astype(f8),
        dwvB8=dwvB8.reshape(128, -1).astype(f8),
        wpT=wpT * 8.0,
        gmask=gmask,
        tmap=tmap,
        eye=_eye_slim(),
    )


def _eye_slim():
    e = np.zeros((128, 24), np.float32)
    for kb in (32, 96):
        for i in range(24):
            e[kb + i, i] = 1.0
    return e


def kernel(x, w_qkv, w_dw, w_proj, temperature, _trace=False):
    import ml_dtypes
    from concourse.bass_utils import run_bass_kernel_spmd

    if "nc" not in _CACHE:
        _CACHE["nc"] = _build()
    nc = _CACHE["nc"]

    consts = _host_consts(w_qkv, w_dw, w_proj, temperature)
    xr = np.ascontiguousarray(np.asarray(x, np.float32).reshape(NCORES, C, HW))
    x8 = xr.astype(ml_dtypes.float8_e4m3)
    xr8 = (xr - x8.astype(np.float32)).astype(ml_dtypes.float8_e4m3)
    in_maps = []
    for bb in range(NCORES):
        m = {"x8": x8[bb], "xr8": xr8[bb],
             "zpad": np.zeros((64, HW), ml_dtypes.float8_e4m3)}
        m.update(consts)
        in_maps.append(m)

    try:
        br = run_bass_kernel_spmd(nc, in_maps, core_ids=list(range(NCORES)),
                                  trace=_trace)
    except ModuleNotFoundError:
        br = run_bass_kernel_spmd(nc, in_maps, core_ids=list(range(NCORES)),
                                  trace=False)
    out = np.stack([np.asarray(r["out"], dtype=np.float32) for r in br.results],
                   axis=0).reshape(NCORES, C, H, W)
    _CACHE["last_results"] = br
    return out



# revision 23
# speedup vs baseline: 1.0560x; 1.0039x over previous
"""Trainium2 Bass kernel for Restormer-style transposed (channel) attention, v2.

Per-core (1 of 8 batch elements), built around the TimelineSim cost model
(matmul cost = output free-size; fp8e4m3 DoubleRow = 0.5 cyc/col):

  q/k path (errors wash out through the softmax normalization):
    z_qk = Wqk8 @ x8            fp8 DoubleRow, 192-contraction in 1 instr
    dwconv 3x3                  5 DR diag tap-pair matmuls per block (2.5 cyc/px)
    -> bf16 qb -> xbar DMA transpose -> [px, slot] qki tiles
    gram G += qki^T qki         bf16, compact-col strided rhs
  v path (kept accurate):
    z_v = Wv @ x16              bf16
    dwconv = DR(fp8(z_v)) + DR(fp8(z_v - fp8(z_v)))   exact to ~0.2%
    vout fp16 resident in SBUF
  tail: norms from gram diag, softmax per head, M^T = A_bd^T Wproj^T,
        out = M @ vout streamed to HBM.

Slot layout (32-aligned, 4 blocks of 128):
  block b: [q_{2b} 0:24 | pad | k_{2b} 32:56 | pad | q_{2b+1} 64:88 | pad |
            k_{2b+1} 96:120 | pad]
"""
import numpy as np

NUM_HEADS = 8
C = 192
H = W = 128
HW = H * W
CD = 24
NCORES = 8
SLAB = 16
NSLABS = H // SLAB
EPS = 1e-12
PW = W + 4
IMG0 = 2
NR = SLAB + 2          # z8 slab rows incl halo

# tap order chosen so DR pairs have EVEN offset deltas (hw requirement):
# pairs: ((-1,-1),(-1,1)) ((0,-1),(0,1)) ((1,-1),(1,1)) ((-1,0),(0,0)) ((1,0),zero)
TAPS = [(-1, -1), (-1, 1), (0, -1), (0, 1), (1, -1), (1, 1), (-1, 0), (0, 0), (1, 0)]

# per-block slot groups: (slot_base, qkv_ch_base)
def _slot_groups(b):
    return [(0, 48 * b), (32, 192 + 48 * b), (64, 48 * b + 24), (96, 192 + 48 * b + 24)]


_CACHE = {}


def _stripes(ncols):
    """split ncols into row-aligned stripes of >=256 (multiples of 128)."""
    out = []
    rem = ncols
    while rem > 0:
        t = min(512, rem)
        if rem - t == 128:
            t = 384
        out.append(t)
        rem -= t
    return out


def _build():
    import concourse.bass as bass
    import concourse.mybir as mybir
    import concourse.tile as tile
    from concourse import bacc
    from contextlib import ExitStack
    import bass_rust

    dt = mybir.dt
    A = mybir.AluOpType
    AF = mybir.ActivationFunctionType
    AX = mybir.AxisListType
    DR = mybir.MatmulPerfMode.DoubleRow
    f32, bf16, f16, f8, f32r = dt.float32, dt.bfloat16, dt.float16, dt.float8e4, dt.float32r

    def ap_dims(ap, dims, extra_offset=0):
        c = ap.copy()
        c.ap = bass_rust.VecI64Pair(dims)
        c.offset = ap.offset + extra_offset
        return c

    nc = bacc.Bacc("TRN2", num_devices=NCORES)

    x8d = nc.dram_tensor("x8", [C, HW], f8, kind="ExternalInput").ap()
    xr8d = nc.dram_tensor("xr8", [C, HW], f8, kind="ExternalInput").ap()
    wq8d = nc.dram_tensor("wq8", [128, 4 * 2 * 128], f8, kind="ExternalInput").ap()
    dwq8d = nc.dram_tensor("dwq8", [128, 4 * 5 * 2 * 128], f8, kind="ExternalInput").ap()
    # v 1x1 weights, out-blocks [128]+[64]: [.., 0:256]=lo j2 m128, [.., 256:384]=hi j2 m64
    wv8d = nc.dram_tensor("wv8", [128, 2 * 128 + 2 * 64], f8, kind="ExternalInput").ap()
    wvr8d = nc.dram_tensor("wvr8", [128, 2 * 128 + 2 * 64], f8, kind="ExternalInput").ap()
    # v dw: A = dual-weight (w8,wr) per tap for ch0-127; rA = tap-paired w8 for r ch0-127;
    # B = mixed block (rows 0-63: z8v ch128-191 dual; rows 64-127: r ch128-191 single w8)
    dwvA8d = nc.dram_tensor("dwvA8", [128, 9 * 2 * 128], f8, kind="ExternalInput").ap()
    dwvrA8d = nc.dram_tensor("dwvrA8", [128, 5 * 2 * 128], f8, kind="ExternalInput").ap()
    dwvB8d = nc.dram_tensor("dwvB8", [128, 9 * 2 * 64], f8, kind="ExternalInput").ap()
    zpadd = nc.dram_tensor("zpad", [64, HW], f8, kind="ExternalInput").ap()
    wpTd = nc.dram_tensor("wpT", [C, C], f32, kind="ExternalInput").ap()
    mskd = nc.dram_tensor("gmask", [128, 4 * 96], f32, kind="ExternalInput").ap()
    tmpd = nc.dram_tensor("tmap", [128, 4], f32, kind="ExternalInput").ap()
    eyed = nc.dram_tensor("eye", [128, 24], f32, kind="ExternalInput").ap()
    outd = nc.dram_tensor("out", [C, HW], f16, kind="ExternalOutput").ap()

    with tile.TileContext(nc) as tc:
      with ExitStack() as _es:
        cpool = _es.enter_context(tc.tile_pool(name="const", bufs=1))
        xpool = _es.enter_context(tc.tile_pool(name="xin", bufs=3))
        zpool = _es.enter_context(tc.tile_pool(name="zst", bufs=2))
        qpool = _es.enter_context(tc.tile_pool(name="qbt", bufs=2))
        kpool = _es.enter_context(tc.tile_pool(name="qki", bufs=2))
        vpool = _es.enter_context(tc.tile_pool(name="vout", bufs=1))
        mpool = _es.enter_context(tc.tile_pool(name="sm", bufs=2))
        apool = _es.enter_context(tc.tile_pool(name="abd", bufs=1))
        opool = _es.enter_context(tc.tile_pool(name="outs", bufs=3))
        psP = _es.enter_context(tc.tile_pool(name="psP", bufs=7, space="PSUM"))
        psG = _es.enter_context(tc.tile_pool(name="psG", bufs=1, space="PSUM"))

        def pstile():
            return psP.tile([128, 512], f32, tag="ps", name="ps")

        # ---------- constants (ACT dma queue; x streams on sync) ----------
        wq8 = cpool.tile([128, 4 * 2 * 128], f8, tag="wq8")
        nc.scalar.dma_start(wq8[:, :], wq8d[:, :])
        wv8 = cpool.tile([128, 2 * 128 + 2 * 64], f8, tag="wv8")
        nc.scalar.dma_start(wv8[:, :], wv8d[:, :])
        wvr8 = cpool.tile([128, 2 * 128 + 2 * 64], f8, tag="wvr8")
        nc.scalar.dma_start(wvr8[:, :], wvr8d[:, :])
        dwq8 = cpool.tile([128, 4 * 5 * 2 * 128], f8, tag="dwq8")
        nc.scalar.dma_start(dwq8[:, :], dwq8d[:, :])
        dwvA8 = cpool.tile([128, 9 * 2 * 128], f8, tag="dwvA8")
        nc.scalar.dma_start(dwvA8[:, :], dwvA8d[:, :])
        dwvrA8 = cpool.tile([128, 5 * 2 * 128], f8, tag="dwvrA8")
        nc.scalar.dma_start(dwvrA8[:, :], dwvrA8d[:, :])
        dwvB8 = cpool.tile([128, 9 * 2 * 64], f8, tag="dwvB8")
        nc.scalar.dma_start(dwvB8[:, :], dwvB8d[:, :])
        msk = cpool.tile([128, 4 * 96], f32, tag="msk")
        nc.scalar.dma_start(msk[:, :], mskd[:, :])
        tmap = cpool.tile([128, 4], f32, tag="tmap")
        nc.scalar.dma_start(tmap[:, :], tmpd[:, :])
        eye = cpool.tile([128, 24], f32, tag="eye")
        nc.scalar.dma_start(eye[:, :], eyed[:, :])
        wp0 = cpool.tile([96, C], f32, tag="wp0")
        nc.scalar.dma_start(wp0[:, :], wpTd[0:96, :])
        wp1 = cpool.tile([96, C], f32, tag="wp1")
        nc.scalar.dma_start(wp1[:, :], wpTd[96:192, :])
        # warm the ACT function tables (Sqrt/Exp) off the critical path
        warm = cpool.tile([1, 2], f32, tag="warm")
        nc.scalar.sqrt(warm[0:1, 0:1], tmap[0:1, 0:1])
        nc.scalar.activation(warm[0:1, 1:2], tmap[0:1, 0:1], AF.Exp,
                             bias=0.0, scale=1.0)

        wq8v = wq8[:, :].rearrange("p (b j s) -> p b j s", b=4, j=2)
        dwq8v = dwq8[:, :].rearrange("p (b k j s) -> p b k j s", b=4, k=5, j=2)
        wv8lo = wv8[:, 0:256].rearrange("p (j s) -> p j s", j=2)
        wv8hi = wv8[:, 256:384].rearrange("p (j s) -> p j s", j=2)
        wvr8lo = wvr8[:, 0:256].rearrange("p (j s) -> p j s", j=2)
        wvr8hi = wvr8[:, 256:384].rearrange("p (j s) -> p j s", j=2)
        dwvA8v = dwvA8[:, :].rearrange("p (t j s) -> p t j s", t=9, j=2)
        dwvrA8v = dwvrA8[:, :].rearrange("p (t j s) -> p t j s", t=5, j=2)
        dwvB8v = dwvB8[:, :].rearrange("p (t j s) -> p t j s", t=9, j=2)

        # vout in fp8 main+residual; contraction j-split 128+64 for M@v DR
        # (j1 rows 64-127 are zero pad, memset once on Pool)
        vout8 = vpool.tile([128, 2 * HW], f8, tag="vout8", name="vout8")
        voutr8 = vpool.tile([128, 2 * HW], f8, tag="voutr8", name="voutr8")
        nc.scalar.dma_start(vout8[64:128, HW:2 * HW], zpadd[:, :])
        nc.scalar.dma_start(voutr8[64:128, HW:2 * HW], zpadd[:, :])
        gram = psG.tile([128, 4 * 96], f32, tag="g")

        # evac engine round-robin (ACT / DVE alternating)
        _ev = [0]
        def cpy(dst, src):
            _ev[0] += 1
            if _ev[0] % 2 == 0:
                nc.scalar.copy(dst, src)
            else:
                nc.vector.tensor_copy(dst, src)

        # slab state carried across pipeline iterations
        z8s, zv8s, qbs, qkis, xts = {}, {}, {}, {}, {}

        def _slabmeta(s):
            r0 = SLAB * s
            lo, hi = max(0, r0 - 1), min(H - 1, r0 + SLAB)
            nrows = hi - lo + 1
            return r0, lo, nrows, lo - (r0 - 1)

        def emit_loads(s, half=None):
            r0, lo, nrows, slot0 = _slabmeta(s)
            ncols = nrows * W
            col0 = lo * W
            if half in (None, 0):
                xs8 = xpool.tile([128, 2 * ncols], f8, tag="xs8")
                nc.sync.dma_start(xs8[:, 0:ncols], x8d[0:128, col0:col0 + ncols])
                # k-tile 1 holds x channels 64..191 (rows 0-63 have zero weights)
                nc.sync.dma_start(xs8[:, ncols:2 * ncols],
                                  x8d[64:192, col0:col0 + ncols])
                xts[s] = [xs8, None, ncols, slot0]
            if half in (None, 1):
                xsr8 = xpool.tile([128, 2 * ncols], f8, tag="xsr8")
                nc.sync.dma_start(xsr8[:, 0:ncols], xr8d[0:128, col0:col0 + ncols])
                nc.sync.dma_start(xsr8[:, ncols:2 * ncols],
                                  xr8d[64:192, col0:col0 + ncols])
                xts[s][1] = xsr8

        def emit_qkv_chunk(s, c):
            r0, lo, nrows, slot0 = _slabmeta(s)
            ncols = nrows * W
            if c == 0:
                z8 = [zpool.tile([128, NR * PW], f8, tag=f"z8_{b}", name=f"z8_{b}")
                      for b in range(4)]
                # zA: z8v ch0-127; rA: r ch0-127; B: rows 0-63 z8v ch128-191,
                # rows 64-127 r ch128-191
                zv8 = [zpool.tile([128, NR * PW], f8, tag=f"zv8_{v}", name=f"zv8_{v}")
                       for v in range(3)]
                if s < 2:
                    for t in z8 + zv8:
                        tv = t[:, :].rearrange("p (r w) -> p r w", w=PW)
                        nc.gpsimd.memset(tv[:, :, 0:IMG0], 0.0)
                        nc.gpsimd.memset(tv[:, :, IMG0 + W:PW], 0.0)
                if s == 0:
                    for t in z8 + zv8:
                        nc.gpsimd.memset(t[:, 0:PW], 0.0)
                if s == NSLABS - 1:
                    for t in z8 + zv8:
                        nc.gpsimd.memset(t[:, (NR - 1) * PW:NR * PW], 0.0)
                z8s[s], zv8s[s] = z8, zv8
            xs8, xsr8, ncols, slot0 = xts[s]
            z8, zv8 = z8s[s], zv8s[s]
            zA, rA, zB = zv8
            xs8p = xs8[:, :].ap[0][0]
            xsr8p = xsr8[:, :].ap[0][0]
            strs = _stripes(ncols)
            todo = [c] if c < 3 else [3] + list(range(4, len(strs)))
            for si in todo:
                tw = strs[si]
                t0 = sum(strs[:si])
                row0 = t0 // W
                nr = tw // W
                for b in range(4):
                  with nc.named_scope(f"qkvqk{s}"):
                    ps = pstile()
                    rhs = ap_dims(xs8[:, :], [[xs8p, 128], [ncols, 2], [1, tw]],
                                  extra_offset=t0)
                    nc.tensor.matmul(ps[0:128, 0:tw], wq8v[:, b, :, :], rhs,
                                     start=True, stop=True, perf_mode=DR)
                    zview = z8[b][:, :].rearrange("p (r w) -> p r w", w=PW)
                    cpy(zview[:, slot0 + row0: slot0 + row0 + nr, IMG0:IMG0 + W],
                        ps[0:128, 0:tw].rearrange("p (r w) -> p r w", w=W))
                with nc.named_scope(f"qkvv{s}"):
                    rhs8 = ap_dims(xs8[:, :], [[xs8p, 128], [ncols, 2], [1, tw]],
                                   extra_offset=t0)
                    rhsr = ap_dims(xsr8[:, :], [[xsr8p, 128], [ncols, 2], [1, tw]],
                                   extra_offset=t0)
                    psl = pstile()
                    nc.tensor.matmul(psl[:, 0:tw], wv8lo, rhs8,
                                     start=True, stop=False, perf_mode=DR)
                    nc.tensor.matmul(psl[:, 0:tw], wv8lo, rhsr,
                                     start=False, stop=False, perf_mode=DR)
                    nc.tensor.matmul(psl[:, 0:tw], wvr8lo, rhs8,
                                     start=False, stop=True, perf_mode=DR)
                    psh0 = pstile()
                    psh = psh0[0:64, 0:tw]
                    nc.tensor.matmul(psh, wv8hi, rhs8,
                                     start=True, stop=False, perf_mode=DR)
                    nc.tensor.matmul(psh, wv8hi, rhsr,
                                     start=False, stop=False, perf_mode=DR)
                    nc.tensor.matmul(psh, wvr8hi, rhs8,
                                     start=False, stop=True, perf_mode=DR)
                    zAv = zA[:, :].rearrange("p (r w) -> p r w", w=PW)
                    rAv = rA[:, :].rearrange("p (r w) -> p r w", w=PW)
                    zBv = zB[:, :].rearrange("p (r w) -> p r w", w=PW)
                    zw = zAv[:, slot0 + row0: slot0 + row0 + nr, IMG0:IMG0 + W]
                    rw = rAv[:, slot0 + row0: slot0 + row0 + nr, IMG0:IMG0 + W]
                    bzw = zBv[0:64, slot0 + row0: slot0 + row0 + nr, IMG0:IMG0 + W]
                    brw = zBv[64:128, slot0 + row0: slot0 + row0 + nr, IMG0:IMG0 + W]
                    pslw = psl[:, 0:tw].rearrange("p (r w) -> p r w", w=W)
                    pshw = psh.rearrange("p (r w) -> p r w", w=W)
                    nc.scalar.copy(zw, pslw)
                    nc.vector.scalar_tensor_tensor(rw, zw, -1.0, pslw, A.mult, A.add)
                    nc.scalar.copy(bzw, pshw)
                    nc.vector.scalar_tensor_tensor(brw, bzw, -1.0, pshw,
                                                   A.mult, A.add)
            if c == 3:
                xts.pop(s)

        def emit_dwqk_chunk(s, c):
            g = c
            if c == 0:
                qbs[s] = [qpool.tile([128, 2 * 4 * W], bf16, tag=f"qb{b}",
                                     name=f"qb{b}") for b in range(4)]
                qkis[s] = [kpool.tile([128, SLAB * W], bf16, tag=f"qki{b}",
                                      name=f"qki{b}") for b in range(4)]
            z8, qb, qki = z8s[s], qbs[s], qkis[s]
            for b in range(4):
              with nc.named_scope(f"dwqk{s}"):
                zp = z8[b][:, :].ap[0][0]
                pd = pstile()
                for p in range(5):
                    dy0, dx0 = TAPS[2 * p]
                    o0 = (4 * g + 1 + dy0) * PW + IMG0 + dx0
                    if 2 * p + 1 < 9:
                        dy1, dx1 = TAPS[2 * p + 1]
                        o1 = (4 * g + 1 + dy1) * PW + IMG0 + dx1
                    else:
                        o1 = o0 + 2
                    rhs = ap_dims(z8[b][:, :],
                                  [[zp, 128], [o1 - o0, 2], [PW, 4], [1, W]],
                                  extra_offset=o0)
                    nc.tensor.matmul(pd[:, :], dwq8v[:, b, p, :, :], rhs,
                                     start=(p == 0), stop=(p == 4),
                                     perf_mode=DR)
                cpy(qb[b][:, (g % 2) * 512:(g % 2) * 512 + 512], pd[:, :])
                if g % 2 == 1:
                    h = g // 2
                    qkv_view = qki[b][:, h * 8 * W:(h + 1) * 8 * W].rearrange(
                        "p (t s) -> p t s", t=8)
                    nc.sync.dma_start_transpose(qkv_view, qb[b][:, :])
            if c == 3:
                z8s.pop(s)

        def emit_dwv_chunk(s, c):
            r0 = SLAB * s
            g = c
            zA, rA, zB = zv8s[s]
            with nc.named_scope(f"dwv{s}"):
                zp = zA[:, :].ap[0][0]
                rp = rA[:, :].ap[0][0]
                bp = zB[:, :].ap[0][0]
                psV = pstile()
                psV20 = pstile()
                psV2 = psV20[0:64, 0:512]
                # A: 9 dual-weight taps (w8 in j0, wr in j1, j-stride 0)
                for t in range(9):
                    dy, dx = TAPS[t]
                    ot = (4 * g + 1 + dy) * PW + IMG0 + dx
                    rhs = ap_dims(zA[:, :],
                                  [[zp, 128], [0, 2], [PW, 4], [1, W]],
                                  extra_offset=ot)
                    nc.tensor.matmul(psV[:, :], dwvA8v[:, t, :, :], rhs,
                                     start=(t == 0), stop=False, perf_mode=DR)
                # rA: 5 tap-paired w8 instrs, accumulate onto psV
                for p in range(5):
                    dy0, dx0 = TAPS[2 * p]
                    o0 = (4 * g + 1 + dy0) * PW + IMG0 + dx0
                    if 2 * p + 1 < 9:
                        dy1, dx1 = TAPS[2 * p + 1]
                        o1 = (4 * g + 1 + dy1) * PW + IMG0 + dx1
                    else:
                        o1 = o0 + 2
                    rhs = ap_dims(rA[:, :],
                                  [[rp, 128], [o1 - o0, 2], [PW, 4], [1, W]],
                                  extra_offset=o0)
                    nc.tensor.matmul(psV[:, :], dwvrA8v[:, p, :, :], rhs,
                                     start=False, stop=(p == 4), perf_mode=DR)
                # B: 9 taps; rows 0-63 dual-weight z8v hi, rows 64-127 r hi (w8)
                for t in range(9):
                    dy, dx = TAPS[t]
                    ot = (4 * g + 1 + dy) * PW + IMG0 + dx
                    rhs = ap_dims(zB[:, :],
                                  [[bp, 128], [0, 2], [PW, 4], [1, W]],
                                  extra_offset=ot)
                    nc.tensor.matmul(psV2, dwvB8v[:, t, :, :], rhs,
                                     start=(t == 0), stop=(t == 8), perf_mode=DR)
                # evac into vout8/voutr8 (j-split 128+64): ch0-127 | ch128-191
                c0 = (r0 + 4 * g) * W
                nc.scalar.copy(vout8[0:128, c0:c0 + 512], psV[:, :])
                nc.vector.scalar_tensor_tensor(
                    voutr8[0:128, c0:c0 + 512], vout8[0:128, c0:c0 + 512],
                    -1.0, psV[:, :], A.mult, A.add)
                nc.scalar.copy(vout8[0:64, HW + c0:HW + c0 + 512], psV2)
                nc.vector.scalar_tensor_tensor(
                    voutr8[0:64, HW + c0:HW + c0 + 512],
                    vout8[0:64, HW + c0:HW + c0 + 512],
                    -1.0, psV2, A.mult, A.add)
            if c == 3:
                zv8s.pop(s)

        def emit_gram_chunk(s, c):
            qki = qkis[s]
            for u in range(4 * c, 4 * c + 4):
                g_idx = SLAB * s + u
                for b in range(4):
                  with nc.named_scope(f"gram{s}"):
                    lhsT = qki[b][:, u * W:(u + 1) * W]
                    rhs = ap_dims(qki[b][:, :],
                                  [[qki[b][:, :].ap[0][0], 128], [32, 4], [1, 24]],
                                  extra_offset=u * W)
                    nc.tensor.matmul(gram[:, 96 * b:96 * (b + 1)], lhsT, rhs,
                                     start=(g_idx == 0), stop=(g_idx == H - 1),
                                     skip_group_check=True)
            if c == 3:
                qbs.pop(s), qkis.pop(s)

        # ---------- pipeline (flat chunk stream with per-stage lags) ----------
        NCHUNK = 4 * NSLABS
        LAGD, LAGG = 2, 6
        emit_loads(0)
        for pos in range(NCHUNK + LAGG + 1):
            s, c = divmod(pos, 4)
            if c in (1, 2) and s + 1 < NSLABS:
                emit_loads(s + 1, half=c - 1)
            if pos < NCHUNK:
                emit_qkv_chunk(s, c)
            p = pos - LAGD
            if 0 <= p < NCHUNK:
                s2, c2 = divmod(p, 4)
                emit_dwqk_chunk(s2, c2)
                if s2 < NSLABS - 1:
                    emit_dwv_chunk(s2, c2)
            p = pos - LAGG
            if 0 <= p < NCHUNK:
                s3, c3 = divmod(p, 4)
                emit_gram_chunk(s3, c3)
                if s3 == NSLABS - 1 and c3 in (2, 3):
                    emit_dwv_chunk(NSLABS - 1, c3 - 2)

        # ---------- norms ----------
        gm = mpool.tile([128, 4 * 96], f32, tag="gm", bufs=1)
        nc.vector.tensor_tensor(gm[:, :], gram[:, :], msk[:, :], A.mult)
        s_sb = mpool.tile([128, 4], f32, tag="ssb")
        nc.vector.tensor_reduce(s_sb[:, :],
                                gm[:, :].rearrange("p (g c) -> p g c", g=4),
                                AX.X, A.add)
        ns = mpool.tile([128, 4], f32, tag="ns")
        nc.scalar.sqrt(ns[:, :], s_sb[:, :])
        nsc = mpool.tile([128, 4], f32, tag="nsc")
        nc.vector.tensor_scalar_max(nsc[:, :], ns[:, :], EPS)
        ry = mpool.tile([128, 4], f32, tag="ry")
        nc.vector.reciprocal(ry[:, :], nsc[:, :])
        t1 = mpool.tile([128, 4], f32, tag="t1")
        nc.vector.tensor_tensor(t1[:, :], s_sb[:, :], ry[:, :], A.mult)
        t2 = mpool.tile([128, 4], f32, tag="t2")
        nc.vector.tensor_add(t2[:, :], nsc[:, :], t1[:, :])
        ns2 = mpool.tile([128, 4], f32, tag="ns2")
        nc.vector.tensor_scalar_mul(ns2[:, :], t2[:, :], 0.5)
        ns3 = mpool.tile([128, 4], f32, tag="ns3")
        nc.vector.tensor_scalar_max(ns3[:, :], ns2[:, :], EPS)
        rn = mpool.tile([128, 4], f32, tag="rn")
        nc.vector.reciprocal(rn[:, :], ns3[:, :])
        rkt = mpool.tile([128, 4], f32, tag="rkt")
        nc.vector.tensor_tensor(rkt[:, :], rn[:, :], tmap[:, :], A.mult)
        rq = mpool.tile([24, 8], f32, tag="rq")
        nc.sync.dma_start(rq[0:24, 1:8:2], rn[64:88, 0:4])

        # ---------- softmax + A blockdiag ----------
        a0 = apool.tile([96, C], f32, tag="a0")
        a1 = apool.tile([96, C], f32, tag="a1")
        nc.vector.memset(a0[:, :], 0.0)
        nc.vector.memset(a1[:, :], 0.0)
        bt = mpool.tile([128, 8 * CD], f32, tag="bt", bufs=1)
        AF_ = AF
        for h in range(NUM_HEADS):
            b = h // 2
            kbase = 32 if h % 2 == 0 else 96
            qcol = 0 if h % 2 == 0 else 48
            nc.vector.tensor_scalar_mul(
                bt[kbase:kbase + CD, CD * h:CD * (h + 1)],
                gram[kbase:kbase + CD, 96 * b + qcol:96 * b + qcol + CD],
                rkt[kbase:kbase + CD, b:b + 1])
            ptr0 = pstile()
            ptr = ptr0[0:CD, 0:CD]
            nc.tensor.transpose(ptr,
                                bt[kbase:kbase + CD, CD * h:CD * (h + 1)],
                                eye[kbase:kbase + CD, 0:CD],
                                tile_position=(kbase, 0))
            es = mpool.tile([CD, CD], f32, tag="es")
            se = mpool.tile([CD, 1], f32, tag="se")
            rqh = (rn[0:24, b:b + 1] if h % 2 == 0
                   else rq[0:24, h:h + 1])
            nc.scalar.activation(es[:, :], ptr, AF_.Exp,
                                 bias=0.0, scale=rqh,
                                 accum_out=se[:, :])
            rse = mpool.tile([CD, 1], f32, tag="rse")
            nc.vector.reciprocal(rse[:, :], se[:, :])
            ah = mpool.tile([CD, CD], f32, tag="ah")
            nc.vector.tensor_scalar_mul(ah[:, :], es[:, :], rse[0:CD, 0:1])
            adst = a0 if h < 4 else a1
            r0 = 24 * (h % 4)
            nc.sync.dma_start(adst[r0:r0 + CD, CD * h:CD * (h + 1)], ah[:, :])

        for c in (2, 3):
            emit_dwv_chunk(NSLABS - 1, c)

        # ---------- M^T = A_bd^T @ (8*W_proj^T) (fp32), then fp8 + residual ----
        # wpT is pre-scaled x8 host-side so M8/Mr stay clear of the fp8
        # subnormal floor; the x8 is undone at the out-evac (scale 1/8).
        # mt8 rows = v-ch: j0 rows 0-127 = ch0-127, j1 rows 0-63 = ch128-191,
        # j1 rows 64-127 zero (matches vout8 pad).
        mt8 = cpool.tile([128, 2 * C], f8, tag="mt8")
        mtr8 = cpool.tile([128, 2 * C], f8, tag="mtr8")
        nc.gpsimd.memset(mt8[64:128, C:2 * C], 0.0)
        nc.gpsimd.memset(mtr8[64:128, C:2 * C], 0.0)
        pmtA0 = pstile()
        pmtA = pmtA0[0:128, 0:C]
        nc.tensor.matmul(pmtA, a0[:, 0:128], wp0[:, :], start=True, stop=False)
        nc.tensor.matmul(pmtA, a1[:, 0:128], wp1[:, :], start=False, stop=True)
        nc.scalar.copy(mt8[0:128, 0:C], pmtA)
        nc.vector.scalar_tensor_tensor(mtr8[0:128, 0:C], mt8[0:128, 0:C], -1.0,
                                       pmtA, A.mult, A.add)
        pmtB0 = pstile()
        pmtB = pmtB0[0:64, 0:C]
        nc.tensor.matmul(pmtB, a0[:, 128:192], wp0[:, :], start=True, stop=False)
        nc.tensor.matmul(pmtB, a1[:, 128:192], wp1[:, :], start=False, stop=True)
        nc.scalar.copy(mt8[0:64, C:2 * C], pmtB)
        nc.vector.scalar_tensor_tensor(mtr8[0:64, C:2 * C], mt8[0:64, C:2 * C],
                                       -1.0, pmtB, A.mult, A.add)
        mt8v = mt8[:, :].rearrange("p (j m) -> p j m", j=2)
        mtr8v = mtr8[:, :].rearrange("p (j m) -> p j m", j=2)

        # ---------- out = M @ v (fp8 DR: M8 v8 + Mr v8 + M8 vr, scaled 1/8) ----
        CHUNK = 1024
        vp = vout8[:, :].ap[0][0]
        vrp = voutr8[:, :].ap[0][0]
        _oe = [0]

        def cpy_scaled(dst, src):
            _oe[0] += 1
            if _oe[0] % 2 == 0:
                nc.scalar.activation(dst, src, AF.Copy, bias=0.0, scale=0.125)
            else:
                nc.vector.tensor_scalar_mul(dst, src, 0.125)

        oa = ob = None
        for t0 in range(0, HW, 512):
          with nc.named_scope("mv"):
            if t0 % CHUNK == 0:
                oa = opool.tile([128, CHUNK], f16, tag="oa")
                ob = opool.tile([64, CHUNK], f16, tag="ob")
            c0 = t0 % CHUNK
            rhs8 = ap_dims(vout8[:, :], [[vp, 128], [HW, 2], [1, 512]],
                           extra_offset=t0)
            rhsr = ap_dims(voutr8[:, :], [[vrp, 128], [HW, 2], [1, 512]],
                           extra_offset=t0)
            pa = pstile()
            nc.tensor.matmul(pa[:, :], mt8v[:, :, 0:128], rhs8,
                             start=True, stop=False, perf_mode=DR)
            nc.tensor.matmul(pa[:, :], mtr8v[:, :, 0:128], rhs8,
                             start=False, stop=False, perf_mode=DR)
            nc.tensor.matmul(pa[:, :], mt8v[:, :, 0:128], rhsr,
                             start=False, stop=True, perf_mode=DR)
            cpy_scaled(oa[:, c0:c0 + 512], pa[:, :])
            pb0 = pstile()
            pb = pb0[0:64, 0:512]
            nc.tensor.matmul(pb, mt8v[:, :, 128:192], rhs8,
                             start=True, stop=False, perf_mode=DR)
            nc.tensor.matmul(pb, mtr8v[:, :, 128:192], rhs8,
                             start=False, stop=False, perf_mode=DR)
            nc.tensor.matmul(pb, mt8v[:, :, 128:192], rhsr,
                             start=False, stop=True, perf_mode=DR)
            cpy_scaled(ob[:, c0:c0 + 512], pb)
            if t0 % CHUNK == CHUNK - 512:
                b0 = t0 + 512 - CHUNK
                nc.sync.dma_start(outd[0:128, b0:b0 + CHUNK], oa[:, :])
                nc.gpsimd.dma_start(outd[128:192, b0:b0 + CHUNK], ob[:, :])

    nc.compile()
    return nc


def _host_consts(w_qkv, w_dw, w_proj, temperature):
    import ml_dtypes
    f8 = ml_dtypes.float8_e4m3

    wq = np.asarray(w_qkv, np.float32)            # [576, 192]
    wd = np.asarray(w_dw, np.float32).reshape(3 * C, 3, 3)
    wpT = np.ascontiguousarray(np.asarray(w_proj, np.float32).T)

    # tap index -> (dy, dx)
    # wq8 [128, 4, 2, 128]
    wq8 = np.zeros((128, 4, 2, 128), np.float32)
    dwq8 = np.zeros((128, 4, 5, 2, 128), np.float32)
    for b in range(4):
        for sb, chb in _slot_groups(b):
            for i in range(CD):
                ch = chb + i
                s = sb + i
                wq8[0:128, b, 0, s] = wq[ch, 0:128]
                wq8[64:128, b, 1, s] = wq[ch, 128:192]
                for t, (dy, dx) in enumerate(TAPS):
                    dwq8[s, b, t // 2, t % 2, s] = wd[ch, dy + 1, dx + 1]

    # v 1x1 weights: out-blocks [128]+[64] packed [128, 2*128 + 2*64]
    wv = np.zeros((128, 2 * 128 + 2 * 64), np.float32)
    for c in range(128):
        ch = 384 + c
        wv[0:128, c] = wq[ch, 0:128]
        wv[64:128, 128 + c] = wq[ch, 128:192]
    for c in range(64):
        ch = 384 + 128 + c
        wv[0:128, 256 + c] = wq[ch, 0:128]
        wv[64:128, 320 + c] = wq[ch, 128:192]
    wv8q = wv.astype(f8)
    wvr8 = (wv - wv8q.astype(np.float32)).astype(f8)

    # v dw weights in fp8 main + residual
    wdv = wd[384:576]                                 # [192, 3, 3]
    wdv_taps = np.stack([wdv[:, dy + 1, dx + 1] for (dy, dx) in TAPS],
                        axis=1)                       # [192, 9]
    w8v = wdv_taps.astype(f8).astype(np.float32)
    wrv = (wdv_taps - w8v).astype(f8).astype(np.float32)
    dwvA8 = np.zeros((128, 9, 2, 128), np.float32)
    dwvrA8 = np.zeros((128, 5, 2, 128), np.float32)
    dwvB8 = np.zeros((128, 9, 2, 64), np.float32)
    for s in range(128):
        for t in range(9):
            dwvA8[s, t, 0, s] = w8v[s, t]
            dwvA8[s, t, 1, s] = wrv[s, t]
        for p in range(5):
            for j in range(2):
                if 2 * p + j < 9:
                    dwvrA8[s, p, j, s] = w8v[s, 2 * p + j]
    for p in range(64):
        ch = 128 + p
        for t in range(9):
            dwvB8[p, t, 0, p] = w8v[ch, t]
            dwvB8[p, t, 1, p] = wrv[ch, t]
            dwvB8[64 + p, t, 0, p] = w8v[ch, t]

    gmask = np.zeros((128, 4 * 96), np.float32)
    for sb, cc in ((0, 0), (32, 24), (64, 48), (96, 72)):
        for i in range(CD):
            for b in range(4):
                gmask[sb + i, 96 * b + cc + i] = 1.0

    tmap = np.ones((128, 4), np.float32)
    tf = np.asarray(temperature, np.float32).reshape(-1)
    for b in range(4):
        tmap[32:56, b] = tf[2 * b]
        tmap[96:120, b] = tf[2 * b + 1]

    return dict(
        wq8=wq8.reshape(128, -1).astype(f8),
        dwq8=dwq8.reshape(128, -1).astype(f8),
        wv8=wv8q,
        wvr8=wvr8,
        dwvA8=dwvA8.reshape(128, -1).astype(f8),
        dwvrA8=dwvrA8.reshape(128, -1).astype(f8),
        dwvB8=dwvB8.reshape(128, -1).astype(f8),
        wpT=wpT * 8.0,
        gmask=gmask,
        tmap=tmap,
        eye=_eye_slim(),
    )


def _eye_slim():
    e = np.zeros((128, 24), np.float32)
    for kb in (32, 96):
        for i in range(24):
            e[kb + i, i] = 1.0
    return e


def kernel(x, w_qkv, w_dw, w_proj, temperature, _trace=False):
    import ml_dtypes
    from concourse.bass_utils import run_bass_kernel_spmd

    if "nc" not in _CACHE:
        _CACHE["nc"] = _build()
    nc = _CACHE["nc"]

    consts = _host_consts(w_qkv, w_dw, w_proj, temperature)
    xr = np.ascontiguousarray(np.asarray(x, np.float32).reshape(NCORES, C, HW))
    x8 = xr.astype(ml_dtypes.float8_e4m3)
    xr8 = (xr - x8.astype(np.float32)).astype(ml_dtypes.float8_e4m3)
    in_maps = []
    for bb in range(NCORES):
        m = {"x8": x8[bb], "xr8": xr8[bb],
             "zpad": np.zeros((64, HW), ml_dtypes.float8_e4m3)}
        m.update(consts)
        in_maps.append(m)

    try:
        br = run_bass_kernel_spmd(nc, in_maps, core_ids=list(range(NCORES)),
                                  trace=_trace)
    except ModuleNotFoundError:
        br = run_bass_kernel_spmd(nc, in_maps, core_ids=list(range(NCORES)),
                                  trace=False)
    out = np.stack([np.asarray(r["out"], dtype=np.float32) for r in br.results],
                   axis=0).reshape(NCORES, C, H, W)
    _CACHE["last_results"] = br
    return out

